# revision 31
# baseline (speedup 1.0000x reference)
"""NeuronSAT GNN message passing on 8 Trainium2 NeuronCores (fp8 edition).

Sharding: data-parallel over graphs - graph g lives entirely on core g.
All state (h, c), weights, and the per-graph bipartite incidence matrices
are SBUF-resident for all 26 rounds.

Layout: feature-major [128=D, nodes]. The literal<->clause aggregations and
the LSTM gate matmuls run in fp8e4m3 with MatmulPerfMode.DoubleRow, which
packs two 128-deep contractions into one PE pass (2 fp8 weights per cell):
- aggregation: incidence-matrix chunk PAIRS contract together
  (7 lit chunks -> 3 DR + 1 plain; 10 clause chunks -> 5 DR)
- gates: the wih/whh (or wihA/wihB) term pairs contract together; the rhs
  is a [128, 2, n] tile holding [agg | h] as contiguous blocks, written
  in place by the aggregation post-op and the LSTM h-update respectively.

All quantization scales are powers of two folded into host-precomputed
weights (lossless in float arithmetic): messages are stored at 64x, h at
128x (|h|<1 so 128|h|<240 never overflows e4m3), gate weights at 256x/128x
so every gate PSUM comes out at 16384x and one activation scale undoes it.
The literal flip (negation) stays a pure column-slice trick by storing
literal h in FLIPPED column order inside the gate-input tile.

Numerics were validated against the jax reference in sim.py: this exact
quantization assignment gives rel err ~5.7e-3 (budget 2e-2).
"""

import sys

sys.path.insert(0, "/opt/trn_rl_repo")

import ml_dtypes
import numpy as np

import concourse.bacc as bacc
import concourse.mybir as mybir
import concourse.tile as tile
from concourse.tile import add_dep_helper
from concourse.bass_utils import run_bass_kernel_spmd

# Problem dims (fixed by the reference).
NG = 8          # graphs == cores
NV = 400        # vars per graph
NCL = 1200      # clauses per graph
KLIT = 5        # literals per clause
NLIT = 2 * NV   # 800 literal nodes per graph
NNG = NLIT + NCL  # 2000 nodes per graph
D = 128
ROUNDS = 26
LCH = 7         # literal 128-chunks (last has 32 rows)
CCH = 10        # clause 128-chunks (last has 48 rows)

F32 = mybir.dt.float32
BF16 = mybir.dt.bfloat16
F8 = mybir.dt.float8e4
NPF8 = ml_dtypes.float8_e4m3fn
NPBF = ml_dtypes.bfloat16
AF = mybir.ActivationFunctionType
ALU = mybir.AluOpType
DR = mybir.MatmulPerfMode.DoubleRow

# Quantization scales (powers of 2; folded into weights host-side).
S_M = 64.0     # messages (m_nm tiles hold 64*m)
S_H = 128.0    # h state (gin block1 holds 128*h; |h|<1 -> <=128 < 240)
S_W = 256.0    # gate weight block0 scale; block1 = S_W*S_M/S_H = 128
S_G = S_M * S_W          # gate psum scale = 16384
INV_G = 1.0 / S_G

# Clause columns: 512-aligned chunks (PSUM bank = 512 f32).
CL_CH = [(0, 512), (512, 512), (1024, 176)]
# Literal columns: 400-wide; flip partner of [0:400] is [400:800].
LIT_CH = [(0, 400), (400, 400)]


def build_nc(rounds=ROUNDS):
    nc = bacc.Bacc(None, target_bir_lowering=False)

    def din(name, shape, dt):
        return nc.declare_dram_parameter(name, list(shape), dt, isOutput=False)

    a_lc_dr_d = din("a_lc_dr", [128, 3, 2, NCL], F8)
    a_lc6_d = din("a_lc6", [128, NCL], F8)
    a_cl_dr_d = din("a_cl_dr", [128, 5, 2, NLIT], F8)
    h0_lit_d = din("h0_lit", [128, NLIT], F8)
    h0_cl_d = din("h0_cl", [128, NCL], F8)
    lmsgT_d = din("lmsgT", [128, 3, 128], BF16)
    lmsg_b_d = din("lmsg_b", [128, 2], F32)
    cmsgT_d = din("cmsgT", [128, 3, 128], BF16)
    cmsg_b_d = din("cmsg_b", [128, 2], F32)
    aggc_b_d = din("aggc_b", [128, 1], F32)
    aggl_b_d = din("aggl_b", [128, NLIT], F32)
    cu_dr_d = din("cu_dr", [128, 4, 2, 128], F8)
    cu_b_d = din("cu_b", [128, 4], F32)
    lu_dr_d = din("lu_dr", [128, 4, 2, 128], F8)
    lu_whhT_d = din("lu_whhT", [128, 4, 128], F8)
    lu_b_d = din("lu_b", [128, 4], F32)
    vw0T_d = din("vw0T", [128, 128], BF16)
    vw1T_d = din("vw1T", [128, 128], BF16)
    vw2T_d = din("vw2T", [128, 1], BF16)
    vb_d = din("vb", [128, 2], F32)

    out_d = nc.declare_dram_parameter("out", [1, 1], F32, isOutput=True)

    with tile.TileContext(nc) as tc:
        with tc.tile_pool(name="singles", bufs=1) as singles, \
             tc.tile_pool(name="work", bufs=2) as work, \
             tc.tile_pool(name="ps", bufs=2, space="PSUM") as psp:

            def load(name, shape, dram, dt):
                t = singles.tile(list(shape), dt, tag=name, name=name)
                nc.sync.dma_start(out=t[:], in_=dram[:])
                return t

            a_lc_dr = load("a_lc_dr", [128, 3, 2, NCL], a_lc_dr_d, F8)
            a_lc6 = load("a_lc6", [128, NCL], a_lc6_d, F8)
            a_cl_dr = load("a_cl_dr", [128, 5, 2, NLIT], a_cl_dr_d, F8)
            lmsgT = load("lmsgT", [128, 3, 128], lmsgT_d, BF16)
            lmsg_b = load("lmsg_b", [128, 2], lmsg_b_d, F32)
            cmsgT = load("cmsgT", [128, 3, 128], cmsgT_d, BF16)
            cmsg_b = load("cmsg_b", [128, 2], cmsg_b_d, F32)
            aggc_b = load("aggc_b", [128, 1], aggc_b_d, F32)
            aggl_b = load("aggl_b", [128, NLIT], aggl_b_d, F32)
            cu_dr = load("cu_dr", [128, 4, 2, 128], cu_dr_d, F8)
            cu_b = load("cu_b", [128, 4], cu_b_d, F32)
            lu_dr = load("lu_dr", [128, 4, 2, 128], lu_dr_d, F8)
            lu_whhT = load("lu_whhT", [128, 4, 128], lu_whhT_d, F8)
            lu_b = load("lu_b", [128, 4], lu_b_d, F32)
            vw0T = load("vw0T", [128, 128], vw0T_d, BF16)
            vw1T = load("vw1T", [128, 128], vw1T_d, BF16)
            vw2T = load("vw2T", [128, 1], vw2T_d, BF16)
            vb = load("vb", [128, 2], vb_d, F32)

            # Gate-input tiles: block0 = agg (64x, fp8), block1 = h (128x,
            # fp8). Lit block1 is stored in FLIPPED column order so the DR
            # rhs [agg_l | h_flip] is one natural 3D slice.
            cl_gin = singles.tile([128, 2, NCL], F8, tag="cl_gin",
                                  name="cl_gin")
            lit_gin = singles.tile([128, 2, NLIT], F8, tag="lit_gin",
                                   name="lit_gin")
            nc.sync.dma_start(out=cl_gin[:, 1, :], in_=h0_cl_d[:])
            nc.sync.dma_start(out=lit_gin[:, 1, :], in_=h0_lit_d[:])

            c_lit = singles.tile([128, NLIT], BF16, tag="c_lit", name="c_lit")
            c_cl = singles.tile([128, NCL], BF16, tag="c_cl", name="c_cl")
            nc.vector.memset(c_lit[:], 0.0)
            nc.vector.memset(c_cl[:], 0.0)

            # Node-major message tiles (fp8, 64x). Fully zeroed once so the
            # never-written tail rows of the last chunks stay 0 (junk fp8
            # bytes could be NaN and 0*NaN = NaN in the DR contraction).
            m_nm = singles.tile([128, 8, 128], F8, tag="m_nmL", name="m_nmL")
            m2_nm = singles.tile([128, 10, 128], F8, tag="m_nmC",
                                 name="m_nmC")
            nc.vector.memset(m_nm[:], 0.0)
            nc.vector.memset(m2_nm[:], 0.0)

            def ps_g(w=512):
                return psp.tile([128, 512], F32, tag="pg", name="pg", bufs=4)

            def ps_m(w=512):
                return psp.tile([128, 512], F32, tag="pm", name="pm", bufs=3)

            pw = psp.tile([128, 256], F32, tag="pW", name="pW", bufs=1)
            warm_rhs = lmsgT[:].rearrange("p a b -> p (a b)")[:, 0:256]

            last_mm = [None]

            def mm(*args, **kw):
                inst = nc.tensor.matmul(*args, **kw)
                last_mm[0] = inst
                return inst

            def pe_warm(n):
                """Keep-warm matmuls (N=256 stream each): harmless PE work
                that keeps the HAM activity window busy so the clock gate
                stays at 2.4GHz. An ordering-only edge to the latest real
                matmul pins the burst at this program position."""
                for k in range(n):
                    d = nc.tensor.matmul(pw[:], vw0T[:], warm_rhs,
                                         start=True, stop=True)
                    if k == 0 and last_mm[0] is not None:
                        add_dep_helper(d.ins, last_mm[0].ins, sync=False,
                                       reason="pin keep-warm burst")

            # ---------------- msg MLP helpers ----------------
            def layer_chunk(dst, srcT, b_ap, c0, w, src_ap, eng="dve"):
                """One MLP layer chunk: matmul + bias/relu post. The post
                runs on ScalarE for layer 1 and DVE for layer 2 so that
                consecutive layers pipeline on different engines."""
                ps = ps_m()
                mm(ps[:, :w], srcT, src_ap, start=True, stop=True)
                if eng == "act":
                    nc.scalar.activation(dst[:, c0:c0 + w], ps[:, :w],
                                         AF.Relu, bias=b_ap)
                else:
                    nc.vector.tensor_scalar(dst[:, c0:c0 + w], ps[:, :w],
                                            b_ap, 0.0, op0=ALU.add,
                                            op1=ALU.max)
                pe_warm(1)

            def nm_group(m_t, x2, ncols, wT2s, g0, gn, eng="dve"):
                """Node-major last-layer chunks g0..g0+gn packed into one
                psum bank, one DVE copy out (cast to fp8; psum is 64x m).
                A final partial chunk (k<128) is copied separately over just
                its valid partitions - the bank's other partitions hold
                stale junk which must not land in the fp8 tile (it is read
                by full-128-partition DR matmuls; fp8 junk can be NaN)."""
                ps = ps_m()
                klast = min(128, ncols - 128 * (g0 + gn - 1))
                for i in range(g0, g0 + gn):
                    k = min(128, ncols - 128 * i)
                    mm(ps[:k, 128 * (i - g0):128 * (i - g0) + 128],
                       x2[:, 128 * i:128 * i + k],
                       wT2s, start=True, stop=True)
                ps3 = ps[:].rearrange("p (b c) -> p b c", c=128)
                nfull = gn if klast == 128 else gn - 1
                if nfull:
                    if eng == "act":
                        nc.scalar.activation(
                            m_t[:, g0:g0 + nfull, :], ps3[:, 0:nfull, :],
                            AF.Copy)
                    else:
                        nc.vector.tensor_copy(m_t[:, g0:g0 + nfull, :],
                                              ps3[:, 0:nfull, :])
                if klast < 128:
                    nc.vector.tensor_copy(
                        m_t[:klast, g0 + gn - 1:g0 + gn, :],
                        ps3[:klast, gn - 1:gn, :])
                pe_warm(1)

            for r in range(rounds):
                # ---- forward: literal message MLP ----
                # L1 reads h (fp8 128x) -> psum 128*(W0@h); bias is 128*b0
                # host-side so x1 = 128*relu(W0@h+b0); W1 is pre-divided by
                # 128 so L2 psum is back at 1x.
                x1 = work.tile([128, NLIT], BF16, tag="mx1", name="mx1",
                               bufs=1)
                x2 = work.tile([128, NLIT], BF16, tag="mx2", name="mx2",
                               bufs=1)
                for (c0, w) in LIT_CH:
                    f0 = (c0 + 400) % 800
                    layer_chunk(x1, lmsgT[:, 0, :], lmsg_b[:, 0:1], c0, w,
                                lit_gin[:, 1, f0:f0 + w], eng="act")
                for (c0, w) in LIT_CH:
                    layer_chunk(x2, lmsgT[:, 1, :], lmsg_b[:, 1:2], c0, w,
                                x1[:, c0:c0 + w], eng="act")
                for (g0, gn) in ((0, 2), (2, 2), (4, 3)):
                    nm_group(m_nm, x2, NLIT, lmsgT[:, 2, :], g0, gn)

                # ---- agg into clauses: 3 DR pairs + 1 plain (32 rows);
                # tail chunk first, posts on ScalarE (idle in this phase) ----
                for (c0, w) in (CL_CH[2], CL_CH[0], CL_CH[1]):
                    ps = ps_m()
                    for g in range(3):
                        mm(ps[:, :w], m_nm[:, 2 * g:2 * g + 2, :],
                           a_lc_dr[:, g, :, c0:c0 + w],
                           start=(g == 0), stop=False, perf_mode=DR)
                    mm(ps[:, :w], m_nm[:32, 6, :], a_lc6[:32, c0:c0 + w],
                       start=False, stop=True)
                    nc.scalar.activation(cl_gin[:, 0, c0:c0 + w], ps[:, :w],
                                         AF.Identity, bias=aggc_b[:, 0:1])
                    pe_warm(1)

                # ---- clause LSTM + C_msg MLP, chunk-major interleaved ----
                cgates = {gi: work.tile([128, NCL], BF16, tag=f"cg{gi}",
                                        name=f"cg{gi}", bufs=1)
                          for gi in range(4)}
                y1 = work.tile([128, NCL], BF16, tag="my1", name="my1",
                               bufs=1)
                y2 = work.tile([128, NCL], BF16, tag="my2", name="my2",
                               bufs=1)

                def cgate(ci):
                    c0, w = CL_CH[ci]
                    for gi in (1, 0, 2, 3):
                        ps = ps_g()
                        mm(ps[:, :w], cu_dr[:, gi, :, :],
                           cl_gin[:, :, c0:c0 + w],
                           start=True, stop=True, perf_mode=DR)
                        fn = AF.Tanh if gi == 2 else AF.Sigmoid
                        nc.scalar.activation(cgates[gi][:, c0:c0 + w],
                                             ps[:, :w], fn,
                                             bias=cu_b[:, gi:gi + 1],
                                             scale=INV_G)
                    pe_warm(2)

                def celem(ci):
                    c0, w = CL_CH[ci]
                    elem_chunk(cgates, c_cl, cl_gin, NCL, c0, c0, w, warm=5)

                def elem_chunk(gates, c_t, gin, n, c0, h0, w, warm):
                    i_ = gates[0][:, c0:c0 + w]
                    f_ = gates[1][:, c0:c0 + w]
                    g_ = gates[2][:, c0:c0 + w]
                    o_ = gates[3][:, c0:c0 + w]
                    cs = c_t[:, c0:c0 + w]
                    t1 = work.tile([128, 512], BF16, tag="t1", name="t1")
                    t2 = work.tile([128, 512], BF16, tag="t2", name="t2")
                    nc.vector.tensor_mul(t1[:, :w], f_, cs)
                    nc.vector.tensor_mul(t2[:, :w], i_, g_)
                    nc.vector.tensor_add(cs, t1[:, :w], t2[:, :w])
                    tc2 = work.tile([128, 512], BF16, tag="tc2", name="tc2")
                    nc.scalar.activation(tc2[:, :w], cs, AF.Tanh)
                    # h (128x, fp8) = (o * 128) * tanh(c2)
                    nc.vector.scalar_tensor_tensor(
                        out=gin[:, 1, h0:h0 + w], in0=o_, scalar=S_H,
                        in1=tc2[:, :w], op0=ALU.mult, op1=ALU.mult)
                    pe_warm(warm)

                def cL1(ci):
                    c0, w = CL_CH[ci]
                    layer_chunk(y1, cmsgT[:, 0, :], cmsg_b[:, 0:1], c0, w,
                                cl_gin[:, 1, c0:c0 + w], eng="act")

                def cL2(ci):
                    c0, w = CL_CH[ci]
                    layer_chunk(y2, cmsgT[:, 1, :], cmsg_b[:, 1:2], c0, w,
                                y1[:, c0:c0 + w], eng="act")

                def cNM(ci):
                    g0, gn = ((0, 2), (2, 2), (4, 2), (6, 2), (8, 2))[ci]
                    nm_group(m2_nm, y2, NCL, cmsgT[:, 2, :], g0, gn)

                cgate(2); cgate(0); celem(2); cgate(1); celem(0)
                cL1(2); celem(1); cL2(2); cL1(0); cNM(4); cL1(1)
                cL2(0); cL2(1); cNM(0); cNM(1); cNM(2); cNM(3)
                pe_warm(5)

                # ---- agg into literals: 5 DR pairs ----
                for (c0, w) in LIT_CH:
                    ps = ps_m()
                    for j in range(5):
                        mm(ps[:, :w], m2_nm[:, 2 * j:2 * j + 2, :],
                           a_cl_dr[:, j, :, c0:c0 + w],
                           start=(j == 0), stop=(j == 4), perf_mode=DR)
                    nc.vector.tensor_add(lit_gin[:, 0, c0:c0 + w],
                                         ps[:, :w], aggl_b[:, c0:c0 + w])
                    pe_warm(1)

                # ---- literal LSTM (gate matmuls all read old h and
                # precede every h write) ----
                lgates = {gi: work.tile([128, NLIT], BF16, tag=f"lg{gi}",
                                        name=f"lg{gi}", bufs=1)
                          for gi in range(4)}

                def lgate(ci):
                    c0, w = LIT_CH[ci]
                    f0 = (c0 + 400) % 800
                    for gi in (1, 0, 2, 3):
                        ps = ps_g()
                        # DR: wihA@agg_l + wihB@h_flip (one natural slice)
                        mm(ps[:, :w], lu_dr[:, gi, :, :],
                           lit_gin[:, :, c0:c0 + w],
                           start=True, stop=False, perf_mode=DR)
                        # plain fp8: whh@h (h of chunk c0 lives at f0)
                        mm(ps[:, :w], lu_whhT[:, gi, :],
                           lit_gin[:, 1, f0:f0 + w],
                           start=False, stop=True)
                        fn = AF.Tanh if gi == 2 else AF.Sigmoid
                        nc.scalar.activation(lgates[gi][:, c0:c0 + w],
                                             ps[:, :w], fn,
                                             bias=lu_b[:, gi:gi + 1],
                                             scale=INV_G)
                    pe_warm(2)

                lgate(0); lgate(1)
                for (c0, w) in LIT_CH:
                    f0 = (c0 + 400) % 800
                    elem_chunk(lgates, c_lit, lit_gin, NLIT, c0, f0, w,
                               warm=6)

            # ---- vote head: mean over literals (sum on device; the flip
            # permutation of columns does not change the sum) ----
            v1 = work.tile([128, NLIT], BF16, tag="v1", name="v1", bufs=1)
            v2 = work.tile([128, NLIT], BF16, tag="v2", name="v2", bufs=1)
            for (c0, w) in LIT_CH:
                ps = ps_m()
                nc.tensor.matmul(ps[:, :w], vw0T[:],
                                 lit_gin[:, 1, c0:c0 + w],
                                 start=True, stop=True)
                nc.scalar.activation(v1[:, c0:c0 + w], ps[:, :w], AF.Relu,
                                     bias=vb[:, 0:1], scale=1.0 / S_H)
            for (c0, w) in LIT_CH:
                ps = ps_m()
                nc.tensor.matmul(ps[:, :w], vw1T[:], v1[:, c0:c0 + w],
                                 start=True, stop=True)
                nc.scalar.activation(v2[:, c0:c0 + w], ps[:, :w], AF.Relu,
                                     bias=vb[:, 1:2])
            acc = work.tile([1, 2], F32, tag="acc", name="acc", bufs=1)
            for ci, (c0, w) in enumerate(LIT_CH):
                ps = ps_m()
                nc.tensor.matmul(ps[0:1, :w], vw2T[:], v2[:, c0:c0 + w],
                                 start=True, stop=True)
                nc.vector.reduce_sum(acc[:, ci:ci + 1], ps[0:1, :w],
                                     axis=mybir.AxisListType.X)
            total = work.tile([1, 1], F32, tag="total", name="total", bufs=1)
            nc.vector.tensor_add(total[:], acc[:, 0:1], acc[:, 1:2])
            nc.sync.dma_start(out=out_d[:], in_=total[:])

    nc.compile()
    return nc


def _f8(x, scale=1.0):
    y = np.asarray(x, np.float32) * scale
    return np.clip(y, -240.0, 240.0).astype(NPF8)


def prep_inputs(inputs):
    """Host-side prep: per-core input dicts from the full problem inputs."""
    f32 = np.float32
    edge_src = np.asarray(inputs["edge_src"]).reshape(NG, NCL * KLIT)
    edge_dst = np.asarray(inputs["edge_dst"]).reshape(NG, NCL * KLIT)

    lmsg_w = np.asarray(inputs["lmsg_w"], f32)
    lmsg_b = np.asarray(inputs["lmsg_b"], f32)
    cmsg_w = np.asarray(inputs["cmsg_w"], f32)
    cmsg_b = np.asarray(inputs["cmsg_b"], f32)

    # msg weight stack [in, layer, out]: layer1 raw, layer2 /128 (to undo
    # the 128x h scale carried into x1), layer3 *64 (message scale); the
    # L1 bias is 128x so x1 = 128*relu(W0@h + b0).
    def msgT(w):
        t = np.transpose(w, (2, 0, 1)).copy()
        t[:, 1, :] /= S_H
        t[:, 2, :] *= S_M
        return np.ascontiguousarray(t).astype(NPBF)

    lmsgT = msgT(lmsg_w)
    cmsgT = msgT(cmsg_w)
    lmsg_b01 = np.ascontiguousarray(lmsg_b[0:2].T * np.float32([[S_H, 1.0]]))
    cmsg_b01 = np.ascontiguousarray(cmsg_b[0:2].T * np.float32([[S_H, 1.0]]))
    # aggregation-side biases carry the 64x message scale
    aggc_b = np.ascontiguousarray((S_M * KLIT * lmsg_b[2])[:, None])

    def gateT(w):  # [512, din] -> [din, 4, 128]
        return np.transpose(np.asarray(w, f32).reshape(4, 128, -1), (2, 0, 1))

    cu_wihT = gateT(inputs["cu_wih"])          # [128, 4, 128]
    cu_whhT = gateT(inputs["cu_whh"])
    cu_dr = _f8(np.stack([cu_wihT * S_W, cu_whhT * (S_W * S_M / S_H)],
                         axis=2))              # [128, 4, 2, 128]
    cu_b = np.ascontiguousarray(
        (np.asarray(inputs["cu_bih"], f32)
         + np.asarray(inputs["cu_bhh"], f32)).reshape(4, 128).T)
    lu_wih = np.asarray(inputs["lu_wih"], f32)  # [512, 256]
    lu_wihTa = gateT(lu_wih[:, :128])
    lu_wihTb = gateT(lu_wih[:, 128:])
    lu_dr = _f8(np.stack([lu_wihTa * S_W, lu_wihTb * (S_W * S_M / S_H)],
                         axis=2))
    lu_whhT = _f8(gateT(inputs["lu_whh"]) * (S_W * S_M / S_H))
    lu_b = np.ascontiguousarray(
        (np.asarray(inputs["lu_bih"], f32)
         + np.asarray(inputs["lu_bhh"], f32)).reshape(4, 128).T)

    vw0T = np.asarray(inputs["vote_w0"], f32).T.astype(NPBF)
    vw1T = np.asarray(inputs["vote_w1"], f32).T.astype(NPBF)
    vw2T = np.asarray(inputs["vote_w2"], f32).T.astype(NPBF)
    vb = np.stack([np.asarray(inputs["vote_b0"], f32),
                   np.asarray(inputs["vote_b1"], f32)], axis=1)

    h0l = (np.asarray(inputs["L_init_w"], f32)[:, 0]
           + np.asarray(inputs["L_init_b"], f32))
    h0c = (np.asarray(inputs["C_init_w"], f32)[:, 0]
           + np.asarray(inputs["C_init_b"], f32))
    h0_lit = _f8(np.broadcast_to(h0l[:, None], (128, NLIT)), S_H)
    h0_cl = _f8(np.broadcast_to(h0c[:, None], (128, NCL)), S_H)

    cmsg_b2 = cmsg_b[2]

    in_maps = []
    for g in range(NG):
        src = edge_src[g] - g * NNG          # local literal ids [0, 800)
        dst = edge_dst[g] - g * NNG - NLIT   # local clause ids [0, 1200)
        A = np.zeros((LCH * 128, NCL), f32)
        np.add.at(A, (src, dst), 1.0)
        deg = A.sum(axis=1)[:NLIT]           # literal degrees
        Ach = A.reshape(LCH, 128, NCL)       # [chunk, row, clause]
        # DR pairs (0,1),(2,3),(4,5) + plain chunk 6
        a_lc_dr = _f8(np.ascontiguousarray(
            Ach[:6].reshape(3, 2, 128, NCL).transpose(2, 0, 1, 3)))
        a_lc6 = _f8(np.ascontiguousarray(Ach[6]))
        At = np.zeros((CCH * 128, NLIT), f32)
        At[:NCL] = A[:NLIT].T
        a_cl_dr = _f8(np.ascontiguousarray(
            At.reshape(5, 2, 128, NLIT).transpose(2, 0, 1, 3)))
        aggl_b = np.ascontiguousarray(S_M * np.outer(cmsg_b2, deg))

        in_maps.append(dict(
            a_lc_dr=a_lc_dr, a_lc6=a_lc6, a_cl_dr=a_cl_dr,
            h0_lit=h0_lit, h0_cl=h0_cl,
            lmsgT=lmsgT, lmsg_b=lmsg_b01, cmsgT=cmsgT, cmsg_b=cmsg_b01,
            aggc_b=aggc_b, aggl_b=aggl_b,
            cu_dr=cu_dr, cu_b=cu_b,
            lu_dr=lu_dr, lu_whhT=lu_whhT, lu_b=lu_b,
            vw0T=vw0T, vw1T=vw1T, vw2T=vw2T, vb=vb,
        ))
    return in_maps


_NC_CACHE = {}
LAST_RESULT = None


def kernel(**inputs):
    global LAST_RESULT
    key = "main"
    if key not in _NC_CACHE:
        _NC_CACHE[key] = build_nc()
    nc = _NC_CACHE[key]
    in_maps = prep_inputs(inputs)
    res = run_bass_kernel_spmd(nc, in_maps, list(range(NG)))
    LAST_RESULT = res
    vote_b2 = float(np.asarray(inputs["vote_b2"], np.float32)[0])
    n_vars = np.asarray(inputs["n_vars"]).astype(np.float32)
    sums = np.array([res.results[g]["out"][0, 0] for g in range(NG)],
                    np.float32)
    sums = sums + np.float32(NLIT * vote_b2)
    return (sums / (2.0 * n_vars)).astype(np.float32)


# revision 32
# speedup vs baseline: 1.0182x; 1.0182x over previous
"""NeuronSAT GNN message passing on 8 Trainium2 NeuronCores (fp8 edition).

Sharding: data-parallel over graphs - graph g lives entirely on core g.
All state (h, c), weights, and the per-graph bipartite incidence matrices
are SBUF-resident for all 26 rounds.

Layout: feature-major [128=D, nodes]. The literal<->clause aggregations and
the LSTM gate matmuls run in fp8e4m3 with MatmulPerfMode.DoubleRow, which
packs two 128-deep contractions into one PE pass (2 fp8 weights per cell):
- aggregation: incidence-matrix chunk PAIRS contract together
  (7 lit chunks -> 3 DR + 1 plain; 10 clause chunks -> 5 DR)
- gates: the wih/whh (or wihA/wihB) term pairs contract together; the rhs
  is a [128, 2, n] tile holding [agg | h] as contiguous blocks, written
  in place by the aggregation post-op and the LSTM h-update respectively.

All quantization scales are powers of two folded into host-precomputed
weights (lossless in float arithmetic): messages are stored at 64x, h at
128x (|h|<1 so 128|h|<240 never overflows e4m3), gate weights at 256x/128x
so every gate PSUM comes out at 16384x and one activation scale undoes it.
The literal flip (negation) stays a pure column-slice trick by storing
literal h in FLIPPED column order inside the gate-input tile.

Numerics were validated against the jax reference in sim.py: this exact
quantization assignment gives rel err ~5.7e-3 (budget 2e-2).
"""

import sys

sys.path.insert(0, "/opt/trn_rl_repo")

import ml_dtypes
import numpy as np

import concourse.bacc as bacc
import concourse.mybir as mybir
import concourse.tile as tile
from concourse.tile import add_dep_helper
from concourse.bass_utils import run_bass_kernel_spmd

# Problem dims (fixed by the reference).
NG = 8          # graphs == cores
NV = 400        # vars per graph
NCL = 1200      # clauses per graph
KLIT = 5        # literals per clause
NLIT = 2 * NV   # 800 literal nodes per graph
NNG = NLIT + NCL  # 2000 nodes per graph
D = 128
ROUNDS = 26
LCH = 7         # literal 128-chunks (last has 32 rows)
CCH = 10        # clause 128-chunks (last has 48 rows)

F32 = mybir.dt.float32
BF16 = mybir.dt.bfloat16
F8 = mybir.dt.float8e4
NPF8 = ml_dtypes.float8_e4m3fn
NPBF = ml_dtypes.bfloat16
AF = mybir.ActivationFunctionType
ALU = mybir.AluOpType
DR = mybir.MatmulPerfMode.DoubleRow

# Quantization scales (powers of 2; folded into weights host-side).
S_M = 64.0     # messages (m_nm tiles hold 64*m)
S_H = 128.0    # h state (gin block1 holds 128*h; |h|<1 -> <=128 < 240)
S_W = 256.0    # gate weight block0 scale; block1 = S_W*S_M/S_H = 128
S_G = S_M * S_W          # gate psum scale = 16384
INV_G = 1.0 / S_G

# Clause columns: 512-aligned chunks (PSUM bank = 512 f32).
CL_CH = [(0, 512), (512, 512), (1024, 176)]
# Literal columns: 400-wide; flip partner of [0:400] is [400:800].
LIT_CH = [(0, 400), (400, 400)]


def build_nc(rounds=ROUNDS):
    nc = bacc.Bacc(None, target_bir_lowering=False)

    def din(name, shape, dt):
        return nc.declare_dram_parameter(name, list(shape), dt, isOutput=False)

    a_lc_dr_d = din("a_lc_dr", [128, 3, 2, NCL], F8)
    a_lc6_d = din("a_lc6", [128, NCL], F8)
    a_cl_dr_d = din("a_cl_dr", [128, 5, 2, NLIT], F8)
    h0_lit_d = din("h0_lit", [128, NLIT], F8)
    h0_cl_d = din("h0_cl", [128, NCL], F8)
    lmsgT_d = din("lmsgT", [128, 3, 128], BF16)
    lmsg_b_d = din("lmsg_b", [128, 2], F32)
    cmsgT_d = din("cmsgT", [128, 3, 128], BF16)
    cmsg_b_d = din("cmsg_b", [128, 2], F32)
    aggc_b_d = din("aggc_b", [128, 1], F32)
    aggl_b_d = din("aggl_b", [128, NLIT], F32)
    cu_dr_d = din("cu_dr", [128, 4, 2, 128], F8)
    cu_b_d = din("cu_b", [128, 4], F32)
    lu_dr_d = din("lu_dr", [128, 4, 2, 128], F8)
    lu_whhT_d = din("lu_whhT", [128, 4, 128], F8)
    lu_b_d = din("lu_b", [128, 4], F32)
    vw0T_d = din("vw0T", [128, 128], BF16)
    vw1T_d = din("vw1T", [128, 128], BF16)
    vw2T_d = din("vw2T", [128, 1], BF16)
    vb_d = din("vb", [128, 2], F32)

    out_d = nc.declare_dram_parameter("out", [1, 1], F32, isOutput=True)

    with tile.TileContext(nc) as tc:
        with tc.tile_pool(name="singles", bufs=1) as singles, \
             tc.tile_pool(name="work", bufs=2) as work, \
             tc.tile_pool(name="ps", bufs=2, space="PSUM") as psp:

            def load(name, shape, dram, dt):
                t = singles.tile(list(shape), dt, tag=name, name=name)
                nc.sync.dma_start(out=t[:], in_=dram[:])
                return t

            a_lc_dr = load("a_lc_dr", [128, 3, 2, NCL], a_lc_dr_d, F8)
            a_lc6 = load("a_lc6", [128, NCL], a_lc6_d, F8)
            a_cl_dr = load("a_cl_dr", [128, 5, 2, NLIT], a_cl_dr_d, F8)
            lmsgT = load("lmsgT", [128, 3, 128], lmsgT_d, BF16)
            lmsg_b = load("lmsg_b", [128, 2], lmsg_b_d, F32)
            cmsgT = load("cmsgT", [128, 3, 128], cmsgT_d, BF16)
            cmsg_b = load("cmsg_b", [128, 2], cmsg_b_d, F32)
            aggc_b = load("aggc_b", [128, 1], aggc_b_d, F32)
            aggl_b = load("aggl_b", [128, NLIT], aggl_b_d, F32)
            cu_dr = load("cu_dr", [128, 4, 2, 128], cu_dr_d, F8)
            cu_b = load("cu_b", [128, 4], cu_b_d, F32)
            lu_dr = load("lu_dr", [128, 4, 2, 128], lu_dr_d, F8)
            lu_whhT = load("lu_whhT", [128, 4, 128], lu_whhT_d, F8)
            lu_b = load("lu_b", [128, 4], lu_b_d, F32)
            vw0T = load("vw0T", [128, 128], vw0T_d, BF16)
            vw1T = load("vw1T", [128, 128], vw1T_d, BF16)
            vw2T = load("vw2T", [128, 1], vw2T_d, BF16)
            vb = load("vb", [128, 2], vb_d, F32)

            # Gate-input tiles: block0 = agg (64x, fp8), block1 = h (128x,
            # fp8). Lit block1 is stored in FLIPPED column order so the DR
            # rhs [agg_l | h_flip] is one natural 3D slice.
            cl_gin = singles.tile([128, 2, NCL], F8, tag="cl_gin",
                                  name="cl_gin")
            lit_gin = singles.tile([128, 2, NLIT], F8, tag="lit_gin",
                                   name="lit_gin")
            nc.sync.dma_start(out=cl_gin[:, 1, :], in_=h0_cl_d[:])
            nc.sync.dma_start(out=lit_gin[:, 1, :], in_=h0_lit_d[:])

            c_lit = singles.tile([128, NLIT], BF16, tag="c_lit", name="c_lit")
            c_cl = singles.tile([128, NCL], BF16, tag="c_cl", name="c_cl")
            nc.vector.memset(c_lit[:], 0.0)
            nc.vector.memset(c_cl[:], 0.0)

            # Node-major message tiles (fp8, 64x). Fully zeroed once so the
            # never-written tail rows of the last chunks stay 0 (junk fp8
            # bytes could be NaN and 0*NaN = NaN in the DR contraction).
            m_nm = singles.tile([128, 8, 128], F8, tag="m_nmL", name="m_nmL")
            m2_nm = singles.tile([128, 10, 128], F8, tag="m_nmC",
                                 name="m_nmC")
            nc.vector.memset(m_nm[:], 0.0)
            nc.vector.memset(m2_nm[:], 0.0)

            def ps_g(w=512):
                return psp.tile([128, 512], F32, tag="pg", name="pg", bufs=4)

            def ps_m(w=512):
                return psp.tile([128, 512], F32, tag="pm", name="pm", bufs=3)

            pw = psp.tile([128, 256], F32, tag="pW", name="pW", bufs=1)
            warm_rhs = lmsgT[:].rearrange("p a b -> p (a b)")[:, 0:256]

            last_mm = [None]

            def mm(*args, **kw):
                inst = nc.tensor.matmul(*args, **kw)
                last_mm[0] = inst
                return inst

            def pe_warm(n):
                """Keep-warm matmuls (N=256 stream each): harmless PE work
                that keeps the HAM activity window busy so the clock gate
                stays at 2.4GHz. An ordering-only edge to the latest real
                matmul pins the burst at this program position."""
                for k in range(n):
                    d = nc.tensor.matmul(pw[:], vw0T[:], warm_rhs,
                                         start=True, stop=True)
                    if k == 0 and last_mm[0] is not None:
                        add_dep_helper(d.ins, last_mm[0].ins, sync=False,
                                       reason="pin keep-warm burst")

            # ---------------- msg MLP helpers ----------------
            def layer_chunk(dst, srcT, b_ap, c0, w, src_ap, eng="dve"):
                """One MLP layer chunk: matmul + bias/relu post. The post
                runs on ScalarE for layer 1 and DVE for layer 2 so that
                consecutive layers pipeline on different engines."""
                ps = ps_m()
                mm(ps[:, :w], srcT, src_ap, start=True, stop=True)
                if eng == "act":
                    nc.scalar.activation(dst[:, c0:c0 + w], ps[:, :w],
                                         AF.Relu, bias=b_ap)
                else:
                    nc.vector.tensor_scalar(dst[:, c0:c0 + w], ps[:, :w],
                                            b_ap, 0.0, op0=ALU.add,
                                            op1=ALU.max)
                pe_warm(1)

            def nm_group(m_t, x2, ncols, wT2s, g0, gn, eng="dve"):
                """Node-major last-layer chunks g0..g0+gn packed into one
                psum bank, one DVE copy out (cast to fp8; psum is 64x m).
                A final partial chunk (k<128) is copied separately over just
                its valid partitions - the bank's other partitions hold
                stale junk which must not land in the fp8 tile (it is read
                by full-128-partition DR matmuls; fp8 junk can be NaN)."""
                ps = ps_m()
                klast = min(128, ncols - 128 * (g0 + gn - 1))
                for i in range(g0, g0 + gn):
                    k = min(128, ncols - 128 * i)
                    mm(ps[:k, 128 * (i - g0):128 * (i - g0) + 128],
                       x2[:, 128 * i:128 * i + k],
                       wT2s, start=True, stop=True)
                ps3 = ps[:].rearrange("p (b c) -> p b c", c=128)
                nfull = gn if klast == 128 else gn - 1
                if nfull:
                    if eng == "act":
                        nc.scalar.activation(
                            m_t[:, g0:g0 + nfull, :], ps3[:, 0:nfull, :],
                            AF.Copy)
                    else:
                        nc.vector.tensor_copy(m_t[:, g0:g0 + nfull, :],
                                              ps3[:, 0:nfull, :])
                if klast < 128:
                    nc.vector.tensor_copy(
                        m_t[:klast, g0 + gn - 1:g0 + gn, :],
                        ps3[:klast, gn - 1:gn, :])
                pe_warm(1)

            for r in range(rounds):
                # ---- forward: literal message MLP ----
                # L1 reads h (fp8 128x) -> psum 128*(W0@h); bias is 128*b0
                # host-side so x1 = 128*relu(W0@h+b0); W1 is pre-divided by
                # 128 so L2 psum is back at 1x.
                x1 = work.tile([128, NLIT], BF16, tag="mx1", name="mx1",
                               bufs=1)
                x2 = work.tile([128, NLIT], BF16, tag="mx2", name="mx2",
                               bufs=1)
                for (c0, w) in LIT_CH:
                    f0 = (c0 + 400) % 800
                    layer_chunk(x1, lmsgT[:, 0, :], lmsg_b[:, 0:1], c0, w,
                                lit_gin[:, 1, f0:f0 + w], eng="act")
                for (c0, w) in LIT_CH:
                    layer_chunk(x2, lmsgT[:, 1, :], lmsg_b[:, 1:2], c0, w,
                                x1[:, c0:c0 + w], eng="act")
                for (g0, gn) in ((0, 2), (2, 2), (4, 3)):
                    nm_group(m_nm, x2, NLIT, lmsgT[:, 2, :], g0, gn)

                # ---- agg into clauses: 3 DR pairs + 1 plain (32 rows);
                # tail chunk first, posts on ScalarE (idle in this phase) ----
                for (c0, w) in (CL_CH[2], CL_CH[0], CL_CH[1]):
                    ps = ps_m()
                    for g in range(3):
                        mm(ps[:, :w], m_nm[:, 2 * g:2 * g + 2, :],
                           a_lc_dr[:, g, :, c0:c0 + w],
                           start=(g == 0), stop=False, perf_mode=DR)
                    mm(ps[:, :w], m_nm[:32, 6, :], a_lc6[:32, c0:c0 + w],
                       start=False, stop=True)
                    nc.scalar.activation(cl_gin[:, 0, c0:c0 + w], ps[:, :w],
                                         AF.Identity, bias=aggc_b[:, 0:1])
                    pe_warm(1)

                # ---- clause LSTM + C_msg MLP, chunk-major interleaved ----
                cgates = {gi: work.tile([128, NCL], BF16, tag=f"cg{gi}",
                                        name=f"cg{gi}", bufs=1)
                          for gi in range(4)}
                y1 = work.tile([128, NCL], BF16, tag="my1", name="my1",
                               bufs=1)
                y2 = work.tile([128, NCL], BF16, tag="my2", name="my2",
                               bufs=1)

                def cgate(ci):
                    c0, w = CL_CH[ci]
                    for gi in (1, 0, 2, 3):
                        ps = ps_g()
                        mm(ps[:, :w], cu_dr[:, gi, :, :],
                           cl_gin[:, :, c0:c0 + w],
                           start=True, stop=True, perf_mode=DR)
                        fn = AF.Tanh if gi == 2 else AF.Sigmoid
                        nc.scalar.activation(cgates[gi][:, c0:c0 + w],
                                             ps[:, :w], fn,
                                             bias=cu_b[:, gi:gi + 1],
                                             scale=INV_G)
                    pe_warm(2)

                def celem(ci):
                    c0, w = CL_CH[ci]
                    elem_chunk(cgates, c_cl, cl_gin, NCL, c0, c0, w, warm=7)

                def elem_chunk(gates, c_t, gin, n, c0, h0, w, warm):
                    i_ = gates[0][:, c0:c0 + w]
                    f_ = gates[1][:, c0:c0 + w]
                    g_ = gates[2][:, c0:c0 + w]
                    o_ = gates[3][:, c0:c0 + w]
                    cs = c_t[:, c0:c0 + w]
                    t1 = work.tile([128, 512], BF16, tag="t1", name="t1")
                    t2 = work.tile([128, 512], BF16, tag="t2", name="t2")
                    nc.vector.tensor_mul(t1[:, :w], f_, cs)
                    nc.vector.tensor_mul(t2[:, :w], i_, g_)
                    nc.vector.tensor_add(cs, t1[:, :w], t2[:, :w])
                    tc2 = work.tile([128, 512], BF16, tag="tc2", name="tc2")
                    nc.scalar.activation(tc2[:, :w], cs, AF.Tanh)
                    # h (128x, fp8) = (o * 128) * tanh(c2)
                    nc.vector.scalar_tensor_tensor(
                        out=gin[:, 1, h0:h0 + w], in0=o_, scalar=S_H,
                        in1=tc2[:, :w], op0=ALU.mult, op1=ALU.mult)
                    pe_warm(warm)

                def cL1(ci):
                    c0, w = CL_CH[ci]
                    layer_chunk(y1, cmsgT[:, 0, :], cmsg_b[:, 0:1], c0, w,
                                cl_gin[:, 1, c0:c0 + w], eng="act")

                def cL2(ci):
                    c0, w = CL_CH[ci]
                    layer_chunk(y2, cmsgT[:, 1, :], cmsg_b[:, 1:2], c0, w,
                                y1[:, c0:c0 + w], eng="act")

                def cNM(ci):
                    g0, gn = ((0, 2), (2, 2), (4, 2), (6, 2), (8, 2))[ci]
                    nm_group(m2_nm, y2, NCL, cmsgT[:, 2, :], g0, gn)

                cgate(2); cgate(0); celem(2); cgate(1); celem(0)
                cL1(2); celem(1); cL2(2); cL1(0); cNM(4); cL1(1)
                cL2(0); cL2(1); cNM(0); cNM(1); cNM(2); cNM(3)
                pe_warm(7)

                # ---- agg into literals: 5 DR pairs ----
                for (c0, w) in LIT_CH:
                    ps = ps_m()
                    for j in range(5):
                        mm(ps[:, :w], m2_nm[:, 2 * j:2 * j + 2, :],
                           a_cl_dr[:, j, :, c0:c0 + w],
                           start=(j == 0), stop=(j == 4), perf_mode=DR)
                    nc.vector.tensor_add(lit_gin[:, 0, c0:c0 + w],
                                         ps[:, :w], aggl_b[:, c0:c0 + w])
                    pe_warm(1)

                # ---- literal LSTM (gate matmuls all read old h and
                # precede every h write) ----
                lgates = {gi: work.tile([128, NLIT], BF16, tag=f"lg{gi}",
                                        name=f"lg{gi}", bufs=1)
                          for gi in range(4)}

                def lgate(ci):
                    c0, w = LIT_CH[ci]
                    f0 = (c0 + 400) % 800
                    for gi in (1, 0, 2, 3):
                        ps = ps_g()
                        # DR: wihA@agg_l + wihB@h_flip (one natural slice)
                        mm(ps[:, :w], lu_dr[:, gi, :, :],
                           lit_gin[:, :, c0:c0 + w],
                           start=True, stop=False, perf_mode=DR)
                        # plain fp8: whh@h (h of chunk c0 lives at f0)
                        mm(ps[:, :w], lu_whhT[:, gi, :],
                           lit_gin[:, 1, f0:f0 + w],
                           start=False, stop=True)
                        fn = AF.Tanh if gi == 2 else AF.Sigmoid
                        nc.scalar.activation(lgates[gi][:, c0:c0 + w],
                                             ps[:, :w], fn,
                                             bias=lu_b[:, gi:gi + 1],
                                             scale=INV_G)
                    pe_warm(2)

                lgate(0); lgate(1)
                for (c0, w) in LIT_CH:
                    f0 = (c0 + 400) % 800
                    elem_chunk(lgates, c_lit, lit_gin, NLIT, c0, f0, w,
                               warm=8)

            # ---- vote head: mean over literals (sum on device; the flip
            # permutation of columns does not change the sum) ----
            v1 = work.tile([128, NLIT], BF16, tag="v1", name="v1", bufs=1)
            v2 = work.tile([128, NLIT], BF16, tag="v2", name="v2", bufs=1)
            for (c0, w) in LIT_CH:
                ps = ps_m()
                nc.tensor.matmul(ps[:, :w], vw0T[:],
                                 lit_gin[:, 1, c0:c0 + w],
                                 start=True, stop=True)
                nc.scalar.activation(v1[:, c0:c0 + w], ps[:, :w], AF.Relu,
                                     bias=vb[:, 0:1], scale=1.0 / S_H)
            for (c0, w) in LIT_CH:
                ps = ps_m()
                nc.tensor.matmul(ps[:, :w], vw1T[:], v1[:, c0:c0 + w],
                                 start=True, stop=True)
                nc.scalar.activation(v2[:, c0:c0 + w], ps[:, :w], AF.Relu,
                                     bias=vb[:, 1:2])
            acc = work.tile([1, 2], F32, tag="acc", name="acc", bufs=1)
            for ci, (c0, w) in enumerate(LIT_CH):
                ps = ps_m()
                nc.tensor.matmul(ps[0:1, :w], vw2T[:], v2[:, c0:c0 + w],
                                 start=True, stop=True)
                nc.vector.reduce_sum(acc[:, ci:ci + 1], ps[0:1, :w],
                                     axis=mybir.AxisListType.X)
            total = work.tile([1, 1], F32, tag="total", name="total", bufs=1)
            nc.vector.tensor_add(total[:], acc[:, 0:1], acc[:, 1:2])
            nc.sync.dma_start(out=out_d[:], in_=total[:])

    nc.compile()
    return nc


def _f8(x, scale=1.0):
    y = np.asarray(x, np.float32) * scale
    return np.clip(y, -240.0, 240.0).astype(NPF8)


def prep_inputs(inputs):
    """Host-side prep: per-core input dicts from the full problem inputs."""
    f32 = np.float32
    edge_src = np.asarray(inputs["edge_src"]).reshape(NG, NCL * KLIT)
    edge_dst = np.asarray(inputs["edge_dst"]).reshape(NG, NCL * KLIT)

    lmsg_w = np.asarray(inputs["lmsg_w"], f32)
    lmsg_b = np.asarray(inputs["lmsg_b"], f32)
    cmsg_w = np.asarray(inputs["cmsg_w"], f32)
    cmsg_b = np.asarray(inputs["cmsg_b"], f32)

    # msg weight stack [in, layer, out]: layer1 raw, layer2 /128 (to undo
    # the 128x h scale carried into x1), layer3 *64 (message scale); the
    # L1 bias is 128x so x1 = 128*relu(W0@h + b0).
    def msgT(w):
        t = np.transpose(w, (2, 0, 1)).copy()
        t[:, 1, :] /= S_H
        t[:, 2, :] *= S_M
        return np.ascontiguousarray(t).astype(NPBF)

    lmsgT = msgT(lmsg_w)
    cmsgT = msgT(cmsg_w)
    lmsg_b01 = np.ascontiguousarray(lmsg_b[0:2].T * np.float32([[S_H, 1.0]]))
    cmsg_b01 = np.ascontiguousarray(cmsg_b[0:2].T * np.float32([[S_H, 1.0]]))
    # aggregation-side biases carry the 64x message scale
    aggc_b = np.ascontiguousarray((S_M * KLIT * lmsg_b[2])[:, None])

    def gateT(w):  # [512, din] -> [din, 4, 128]
        return np.transpose(np.asarray(w, f32).reshape(4, 128, -1), (2, 0, 1))

    cu_wihT = gateT(inputs["cu_wih"])          # [128, 4, 128]
    cu_whhT = gateT(inputs["cu_whh"])
    cu_dr = _f8(np.stack([cu_wihT * S_W, cu_whhT * (S_W * S_M / S_H)],
                         axis=2))              # [128, 4, 2, 128]
    cu_b = np.ascontiguousarray(
        (np.asarray(inputs["cu_bih"], f32)
         + np.asarray(inputs["cu_bhh"], f32)).reshape(4, 128).T)
    lu_wih = np.asarray(inputs["lu_wih"], f32)  # [512, 256]
    lu_wihTa = gateT(lu_wih[:, :128])
    lu_wihTb = gateT(lu_wih[:, 128:])
    lu_dr = _f8(np.stack([lu_wihTa * S_W, lu_wihTb * (S_W * S_M / S_H)],
                         axis=2))
    lu_whhT = _f8(gateT(inputs["lu_whh"]) * (S_W * S_M / S_H))
    lu_b = np.ascontiguousarray(
        (np.asarray(inputs["lu_bih"], f32)
         + np.asarray(inputs["lu_bhh"], f32)).reshape(4, 128).T)

    vw0T = np.asarray(inputs["vote_w0"], f32).T.astype(NPBF)
    vw1T = np.asarray(inputs["vote_w1"], f32).T.astype(NPBF)
    vw2T = np.asarray(inputs["vote_w2"], f32).T.astype(NPBF)
    vb = np.stack([np.asarray(inputs["vote_b0"], f32),
                   np.asarray(inputs["vote_b1"], f32)], axis=1)

    h0l = (np.asarray(inputs["L_init_w"], f32)[:, 0]
           + np.asarray(inputs["L_init_b"], f32))
    h0c = (np.asarray(inputs["C_init_w"], f32)[:, 0]
           + np.asarray(inputs["C_init_b"], f32))
    h0_lit = _f8(np.broadcast_to(h0l[:, None], (128, NLIT)), S_H)
    h0_cl = _f8(np.broadcast_to(h0c[:, None], (128, NCL)), S_H)

    cmsg_b2 = cmsg_b[2]

    in_maps = []
    for g in range(NG):
        src = edge_src[g] - g * NNG          # local literal ids [0, 800)
        dst = edge_dst[g] - g * NNG - NLIT   # local clause ids [0, 1200)
        A = np.zeros((LCH * 128, NCL), f32)
        np.add.at(A, (src, dst), 1.0)
        deg = A.sum(axis=1)[:NLIT]           # literal degrees
        Ach = A.reshape(LCH, 128, NCL)       # [chunk, row, clause]
        # DR pairs (0,1),(2,3),(4,5) + plain chunk 6
        a_lc_dr = _f8(np.ascontiguousarray(
            Ach[:6].reshape(3, 2, 128, NCL).transpose(2, 0, 1, 3)))
        a_lc6 = _f8(np.ascontiguousarray(Ach[6]))
        At = np.zeros((CCH * 128, NLIT), f32)
        At[:NCL] = A[:NLIT].T
        a_cl_dr = _f8(np.ascontiguousarray(
            At.reshape(5, 2, 128, NLIT).transpose(2, 0, 1, 3)))
        aggl_b = np.ascontiguousarray(S_M * np.outer(cmsg_b2, deg))

        in_maps.append(dict(
            a_lc_dr=a_lc_dr, a_lc6=a_lc6, a_cl_dr=a_cl_dr,
            h0_lit=h0_lit, h0_cl=h0_cl,
            lmsgT=lmsgT, lmsg_b=lmsg_b01, cmsgT=cmsgT, cmsg_b=cmsg_b01,
            aggc_b=aggc_b, aggl_b=aggl_b,
            cu_dr=cu_dr, cu_b=cu_b,
            lu_dr=lu_dr, lu_whhT=lu_whhT, lu_b=lu_b,
            vw0T=vw0T, vw1T=vw1T, vw2T=vw2T, vb=vb,
        ))
    return in_maps


_NC_CACHE = {}
LAST_RESULT = None


def kernel(**inputs):
    global LAST_RESULT
    key = "main"
    if key not in _NC_CACHE:
        _NC_CACHE[key] = build_nc()
    nc = _NC_CACHE[key]
    in_maps = prep_inputs(inputs)
    res = run_bass_kernel_spmd(nc, in_maps, list(range(NG)))
    LAST_RESULT = res
    vote_b2 = float(np.asarray(inputs["vote_b2"], np.float32)[0])
    n_vars = np.asarray(inputs["n_vars"]).astype(np.float32)
    sums = np.array([res.results[g]["out"][0, 0] for g in range(NG)],
                    np.float32)
    sums = sums + np.float32(NLIT * vote_b2)
    return (sums / (2.0 * n_vars)).astype(np.float32)


# revision 33
# speedup vs baseline: 1.0206x; 1.0023x over previous
"""NeuronSAT GNN message passing on 8 Trainium2 NeuronCores (fp8 edition).

Sharding: data-parallel over graphs - graph g lives entirely on core g.
All state (h, c), weights, and the per-graph bipartite incidence matrices
are SBUF-resident for all 26 rounds.

Layout: feature-major [128=D, nodes]. The literal<->clause aggregations and
the LSTM gate matmuls run in fp8e4m3 with MatmulPerfMode.DoubleRow, which
packs two 128-deep contractions into one PE pass (2 fp8 weights per cell):
- aggregation: incidence-matrix chunk PAIRS contract together
  (7 lit chunks -> 3 DR + 1 plain; 10 clause chunks -> 5 DR)
- gates: the wih/whh (or wihA/wihB) term pairs contract together; the rhs
  is a [128, 2, n] tile holding [agg | h] as contiguous blocks, written
  in place by the aggregation post-op and the LSTM h-update respectively.

All quantization scales are powers of two folded into host-precomputed
weights (lossless in float arithmetic): messages are stored at 64x, h at
128x (|h|<1 so 128|h|<240 never overflows e4m3), gate weights at 256x/128x
so every gate PSUM comes out at 16384x and one activation scale undoes it.
The literal flip (negation) stays a pure column-slice trick by storing
literal h in FLIPPED column order inside the gate-input tile.

Numerics were validated against the jax reference in sim.py: this exact
quantization assignment gives rel err ~6.2e-3 on hardware (budget 2e-2).

Schedule notes (evidence-driven, from perfetto/NTFF traces):
- The round is a serial chain of 8 phases (two LSTM sides x MLP/agg/
  gates/elem); engine assignment is balanced per phase: all msg-MLP
  bias+relu posts ride ScalarE (idle during MLP phases while DVE does the
  node-major casts), agg_c posts ride ScalarE, agg_l posts (full-tensor
  bias) and all LSTM elementwise ride DVE.
- The 176-wide clause tail chunk is processed FIRST so its
  gates/elem/MLP/cast chain completes early and the last node-major cast
  (which gates agg_l) lands sooner.
- pe_warm keeps the PE HAM activity window busy (idle >3.4us throttles
  the PE clock to 1.2GHz); counts are tuned so throttle stays ~10-20us
  total without crowding real matmuls out of the queue (measured optimum;
  more warm delays the round-boundary chain, less re-throttles).
- Pair-major aggregation orders and 2-bank-wide gate ACTs were tried and
  measured SLOWER (psum rotation serialization / delayed elem starts).
HW exec time: ~726us on trn2 (baseline bf16 kernel: ~878us).
"""

import sys

sys.path.insert(0, "/opt/trn_rl_repo")

import ml_dtypes
import numpy as np

import concourse.bacc as bacc
import concourse.mybir as mybir
import concourse.tile as tile
from concourse.tile import add_dep_helper
from concourse.bass_utils import run_bass_kernel_spmd

# Problem dims (fixed by the reference).
NG = 8          # graphs == cores
NV = 400        # vars per graph
NCL = 1200      # clauses per graph
KLIT = 5        # literals per clause
NLIT = 2 * NV   # 800 literal nodes per graph
NNG = NLIT + NCL  # 2000 nodes per graph
D = 128
ROUNDS = 26
LCH = 7         # literal 128-chunks (last has 32 rows)
CCH = 10        # clause 128-chunks (last has 48 rows)

F32 = mybir.dt.float32
BF16 = mybir.dt.bfloat16
F8 = mybir.dt.float8e4
NPF8 = ml_dtypes.float8_e4m3fn
NPBF = ml_dtypes.bfloat16
AF = mybir.ActivationFunctionType
ALU = mybir.AluOpType
DR = mybir.MatmulPerfMode.DoubleRow

# Quantization scales (powers of 2; folded into weights host-side).
S_M = 64.0     # messages (m_nm tiles hold 64*m)
S_H = 128.0    # h state (gin block1 holds 128*h; |h|<1 -> <=128 < 240)
S_W = 256.0    # gate weight block0 scale; block1 = S_W*S_M/S_H = 128
S_G = S_M * S_W          # gate psum scale = 16384
INV_G = 1.0 / S_G

# Clause columns: 512-aligned chunks (PSUM bank = 512 f32).
CL_CH = [(0, 512), (512, 512), (1024, 176)]
# Literal columns: 400-wide; flip partner of [0:400] is [400:800].
LIT_CH = [(0, 400), (400, 400)]


def build_nc(rounds=ROUNDS):
    nc = bacc.Bacc(None, target_bir_lowering=False)

    def din(name, shape, dt):
        return nc.declare_dram_parameter(name, list(shape), dt, isOutput=False)

    a_lc_dr_d = din("a_lc_dr", [128, 3, 2, NCL], F8)
    a_lc6_d = din("a_lc6", [128, NCL], F8)
    a_cl_dr_d = din("a_cl_dr", [128, 5, 2, NLIT], F8)
    h0_lit_d = din("h0_lit", [128, NLIT], F8)
    h0_cl_d = din("h0_cl", [128, NCL], F8)
    lmsgT_d = din("lmsgT", [128, 3, 128], BF16)
    lmsg_b_d = din("lmsg_b", [128, 2], F32)
    cmsgT_d = din("cmsgT", [128, 3, 128], BF16)
    cmsg_b_d = din("cmsg_b", [128, 2], F32)
    aggc_b_d = din("aggc_b", [128, 1], F32)
    aggl_b_d = din("aggl_b", [128, NLIT], F32)
    cu_dr_d = din("cu_dr", [128, 4, 2, 128], F8)
    cu_b_d = din("cu_b", [128, 4], F32)
    lu_dr_d = din("lu_dr", [128, 4, 2, 128], F8)
    lu_whhT_d = din("lu_whhT", [128, 4, 128], F8)
    lu_b_d = din("lu_b", [128, 4], F32)
    vw0T_d = din("vw0T", [128, 128], BF16)
    vw1T_d = din("vw1T", [128, 128], BF16)
    vw2T_d = din("vw2T", [128, 1], BF16)
    vb_d = din("vb", [128, 2], F32)

    out_d = nc.declare_dram_parameter("out", [1, 1], F32, isOutput=True)

    with tile.TileContext(nc) as tc:
        with tc.tile_pool(name="singles", bufs=1) as singles, \
             tc.tile_pool(name="work", bufs=2) as work, \
             tc.tile_pool(name="ps", bufs=2, space="PSUM") as psp:

            def load(name, shape, dram, dt):
                t = singles.tile(list(shape), dt, tag=name, name=name)
                nc.sync.dma_start(out=t[:], in_=dram[:])
                return t

            a_lc_dr = load("a_lc_dr", [128, 3, 2, NCL], a_lc_dr_d, F8)
            a_lc6 = load("a_lc6", [128, NCL], a_lc6_d, F8)
            a_cl_dr = load("a_cl_dr", [128, 5, 2, NLIT], a_cl_dr_d, F8)
            lmsgT = load("lmsgT", [128, 3, 128], lmsgT_d, BF16)
            lmsg_b = load("lmsg_b", [128, 2], lmsg_b_d, F32)
            cmsgT = load("cmsgT", [128, 3, 128], cmsgT_d, BF16)
            cmsg_b = load("cmsg_b", [128, 2], cmsg_b_d, F32)
            aggc_b = load("aggc_b", [128, 1], aggc_b_d, F32)
            aggl_b = load("aggl_b", [128, NLIT], aggl_b_d, F32)
            cu_dr = load("cu_dr", [128, 4, 2, 128], cu_dr_d, F8)
            cu_b = load("cu_b", [128, 4], cu_b_d, F32)
            lu_dr = load("lu_dr", [128, 4, 2, 128], lu_dr_d, F8)
            lu_whhT = load("lu_whhT", [128, 4, 128], lu_whhT_d, F8)
            lu_b = load("lu_b", [128, 4], lu_b_d, F32)
            vw0T = load("vw0T", [128, 128], vw0T_d, BF16)
            vw1T = load("vw1T", [128, 128], vw1T_d, BF16)
            vw2T = load("vw2T", [128, 1], vw2T_d, BF16)
            vb = load("vb", [128, 2], vb_d, F32)

            # Gate-input tiles: block0 = agg (64x, fp8), block1 = h (128x,
            # fp8). Lit block1 is stored in FLIPPED column order so the DR
            # rhs [agg_l | h_flip] is one natural 3D slice.
            cl_gin = singles.tile([128, 2, NCL], F8, tag="cl_gin",
                                  name="cl_gin")
            lit_gin = singles.tile([128, 2, NLIT], F8, tag="lit_gin",
                                   name="lit_gin")
            nc.sync.dma_start(out=cl_gin[:, 1, :], in_=h0_cl_d[:])
            nc.sync.dma_start(out=lit_gin[:, 1, :], in_=h0_lit_d[:])

            c_lit = singles.tile([128, NLIT], BF16, tag="c_lit", name="c_lit")
            c_cl = singles.tile([128, NCL], BF16, tag="c_cl", name="c_cl")
            nc.vector.memset(c_lit[:], 0.0)
            nc.vector.memset(c_cl[:], 0.0)

            # Node-major message tiles (fp8, 64x). Fully zeroed once so the
            # never-written tail rows of the last chunks stay 0 (junk fp8
            # bytes could be NaN and 0*NaN = NaN in the DR contraction).
            m_nm = singles.tile([128, 8, 128], F8, tag="m_nmL", name="m_nmL")
            m2_nm = singles.tile([128, 10, 128], F8, tag="m_nmC",
                                 name="m_nmC")
            nc.vector.memset(m_nm[:], 0.0)
            nc.vector.memset(m2_nm[:], 0.0)

            def ps_g(w=512):
                return psp.tile([128, 512], F32, tag="pg", name="pg", bufs=4)

            def ps_m(w=512):
                return psp.tile([128, 512], F32, tag="pm", name="pm", bufs=3)

            pw = psp.tile([128, 256], F32, tag="pW", name="pW", bufs=1)
            warm_rhs = lmsgT[:].rearrange("p a b -> p (a b)")[:, 0:256]

            last_mm = [None]

            def mm(*args, **kw):
                inst = nc.tensor.matmul(*args, **kw)
                last_mm[0] = inst
                return inst

            def pe_warm(n):
                """Keep-warm matmuls (N=256 stream each): harmless PE work
                that keeps the HAM activity window busy so the clock gate
                stays at 2.4GHz. An ordering-only edge to the latest real
                matmul pins the burst at this program position."""
                for k in range(n):
                    d = nc.tensor.matmul(pw[:], vw0T[:], warm_rhs,
                                         start=True, stop=True)
                    if k == 0 and last_mm[0] is not None:
                        add_dep_helper(d.ins, last_mm[0].ins, sync=False,
                                       reason="pin keep-warm burst")

            # ---------------- msg MLP helpers ----------------
            def layer_chunk(dst, srcT, b_ap, c0, w, src_ap, eng="dve"):
                """One MLP layer chunk: matmul + bias/relu post. The post
                runs on ScalarE for layer 1 and DVE for layer 2 so that
                consecutive layers pipeline on different engines."""
                ps = ps_m()
                mm(ps[:, :w], srcT, src_ap, start=True, stop=True)
                if eng == "act":
                    nc.scalar.activation(dst[:, c0:c0 + w], ps[:, :w],
                                         AF.Relu, bias=b_ap)
                else:
                    nc.vector.tensor_scalar(dst[:, c0:c0 + w], ps[:, :w],
                                            b_ap, 0.0, op0=ALU.add,
                                            op1=ALU.max)
                pe_warm(1)

            def nm_group(m_t, x2, ncols, wT2s, g0, gn, eng="dve"):
                """Node-major last-layer chunks g0..g0+gn packed into one
                psum bank, one DVE copy out (cast to fp8; psum is 64x m).
                A final partial chunk (k<128) is copied separately over just
                its valid partitions - the bank's other partitions hold
                stale junk which must not land in the fp8 tile (it is read
                by full-128-partition DR matmuls; fp8 junk can be NaN)."""
                ps = ps_m()
                klast = min(128, ncols - 128 * (g0 + gn - 1))
                for i in range(g0, g0 + gn):
                    k = min(128, ncols - 128 * i)
                    mm(ps[:k, 128 * (i - g0):128 * (i - g0) + 128],
                       x2[:, 128 * i:128 * i + k],
                       wT2s, start=True, stop=True)
                ps3 = ps[:].rearrange("p (b c) -> p b c", c=128)
                nfull = gn if klast == 128 else gn - 1
                if nfull:
                    if eng == "act":
                        nc.scalar.activation(
                            m_t[:, g0:g0 + nfull, :], ps3[:, 0:nfull, :],
                            AF.Copy)
                    else:
                        nc.vector.tensor_copy(m_t[:, g0:g0 + nfull, :],
                                              ps3[:, 0:nfull, :])
                if klast < 128:
                    nc.vector.tensor_copy(
                        m_t[:klast, g0 + gn - 1:g0 + gn, :],
                        ps3[:klast, gn - 1:gn, :])
                pe_warm(1)

            for r in range(rounds):
                # ---- forward: literal message MLP ----
                # L1 reads h (fp8 128x) -> psum 128*(W0@h); bias is 128*b0
                # host-side so x1 = 128*relu(W0@h+b0); W1 is pre-divided by
                # 128 so L2 psum is back at 1x.
                x1 = work.tile([128, NLIT], BF16, tag="mx1", name="mx1",
                               bufs=1)
                x2 = work.tile([128, NLIT], BF16, tag="mx2", name="mx2",
                               bufs=1)
                for (c0, w) in LIT_CH:
                    f0 = (c0 + 400) % 800
                    layer_chunk(x1, lmsgT[:, 0, :], lmsg_b[:, 0:1], c0, w,
                                lit_gin[:, 1, f0:f0 + w], eng="act")
                for (c0, w) in LIT_CH:
                    layer_chunk(x2, lmsgT[:, 1, :], lmsg_b[:, 1:2], c0, w,
                                x1[:, c0:c0 + w], eng="act")
                for (g0, gn) in ((0, 2), (2, 2), (4, 3)):
                    nm_group(m_nm, x2, NLIT, lmsgT[:, 2, :], g0, gn)

                # ---- agg into clauses: 3 DR pairs + 1 plain (32 rows);
                # tail chunk first, posts on ScalarE (idle in this phase) ----
                for (c0, w) in (CL_CH[2], CL_CH[0], CL_CH[1]):
                    ps = ps_m()
                    for g in range(3):
                        mm(ps[:, :w], m_nm[:, 2 * g:2 * g + 2, :],
                           a_lc_dr[:, g, :, c0:c0 + w],
                           start=(g == 0), stop=False, perf_mode=DR)
                    mm(ps[:, :w], m_nm[:32, 6, :], a_lc6[:32, c0:c0 + w],
                       start=False, stop=True)
                    nc.scalar.activation(cl_gin[:, 0, c0:c0 + w], ps[:, :w],
                                         AF.Identity, bias=aggc_b[:, 0:1])
                    pe_warm(1)

                # ---- clause LSTM + C_msg MLP, chunk-major interleaved ----
                cgates = {gi: work.tile([128, NCL], BF16, tag=f"cg{gi}",
                                        name=f"cg{gi}", bufs=1)
                          for gi in range(4)}
                y1 = work.tile([128, NCL], BF16, tag="my1", name="my1",
                               bufs=1)
                y2 = work.tile([128, NCL], BF16, tag="my2", name="my2",
                               bufs=1)

                def cgate(ci):
                    c0, w = CL_CH[ci]
                    for gi in (1, 0, 2, 3):
                        ps = ps_g()
                        mm(ps[:, :w], cu_dr[:, gi, :, :],
                           cl_gin[:, :, c0:c0 + w],
                           start=True, stop=True, perf_mode=DR)
                        fn = AF.Tanh if gi == 2 else AF.Sigmoid
                        nc.scalar.activation(cgates[gi][:, c0:c0 + w],
                                             ps[:, :w], fn,
                                             bias=cu_b[:, gi:gi + 1],
                                             scale=INV_G)
                    pe_warm(2)

                def celem(ci):
                    c0, w = CL_CH[ci]
                    elem_chunk(cgates, c_cl, cl_gin, NCL, c0, c0, w, warm=7)

                def elem_chunk(gates, c_t, gin, n, c0, h0, w, warm):
                    i_ = gates[0][:, c0:c0 + w]
                    f_ = gates[1][:, c0:c0 + w]
                    g_ = gates[2][:, c0:c0 + w]
                    o_ = gates[3][:, c0:c0 + w]
                    cs = c_t[:, c0:c0 + w]
                    t1 = work.tile([128, 512], BF16, tag="t1", name="t1")
                    t2 = work.tile([128, 512], BF16, tag="t2", name="t2")
                    nc.vector.tensor_mul(t1[:, :w], f_, cs)
                    nc.vector.tensor_mul(t2[:, :w], i_, g_)
                    nc.vector.tensor_add(cs, t1[:, :w], t2[:, :w])
                    tc2 = work.tile([128, 512], BF16, tag="tc2", name="tc2")
                    nc.scalar.activation(tc2[:, :w], cs, AF.Tanh)
                    # h (128x, fp8) = (o * 128) * tanh(c2)
                    nc.vector.scalar_tensor_tensor(
                        out=gin[:, 1, h0:h0 + w], in0=o_, scalar=S_H,
                        in1=tc2[:, :w], op0=ALU.mult, op1=ALU.mult)
                    pe_warm(warm)

                def cL1(ci):
                    c0, w = CL_CH[ci]
                    layer_chunk(y1, cmsgT[:, 0, :], cmsg_b[:, 0:1], c0, w,
                                cl_gin[:, 1, c0:c0 + w], eng="act")

                def cL2(ci):
                    c0, w = CL_CH[ci]
                    layer_chunk(y2, cmsgT[:, 1, :], cmsg_b[:, 1:2], c0, w,
                                y1[:, c0:c0 + w], eng="act")

                def cNM(ci):
                    g0, gn = ((0, 2), (2, 2), (4, 2), (6, 2), (8, 2))[ci]
                    nm_group(m2_nm, y2, NCL, cmsgT[:, 2, :], g0, gn)

                cgate(2); cgate(0); celem(2); cgate(1); celem(0)
                cL1(2); celem(1); cL2(2); cL1(0); cNM(4); cL1(1)
                cL2(0); cL2(1); cNM(0); cNM(1); cNM(2); cNM(3)
                pe_warm(7)

                # ---- agg into literals: 5 DR pairs ----
                for (c0, w) in LIT_CH:
                    ps = ps_m()
                    for j in range(5):
                        mm(ps[:, :w], m2_nm[:, 2 * j:2 * j + 2, :],
                           a_cl_dr[:, j, :, c0:c0 + w],
                           start=(j == 0), stop=(j == 4), perf_mode=DR)
                    nc.vector.tensor_add(lit_gin[:, 0, c0:c0 + w],
                                         ps[:, :w], aggl_b[:, c0:c0 + w])
                    pe_warm(1)

                # ---- literal LSTM (gate matmuls all read old h and
                # precede every h write) ----
                lgates = {gi: work.tile([128, NLIT], BF16, tag=f"lg{gi}",
                                        name=f"lg{gi}", bufs=1)
                          for gi in range(4)}

                def lgate(ci):
                    c0, w = LIT_CH[ci]
                    f0 = (c0 + 400) % 800
                    for gi in (1, 0, 2, 3):
                        ps = ps_g()
                        # DR: wihA@agg_l + wihB@h_flip (one natural slice)
                        mm(ps[:, :w], lu_dr[:, gi, :, :],
                           lit_gin[:, :, c0:c0 + w],
                           start=True, stop=False, perf_mode=DR)
                        # plain fp8: whh@h (h of chunk c0 lives at f0)
                        mm(ps[:, :w], lu_whhT[:, gi, :],
                           lit_gin[:, 1, f0:f0 + w],
                           start=False, stop=True)
                        fn = AF.Tanh if gi == 2 else AF.Sigmoid
                        nc.scalar.activation(lgates[gi][:, c0:c0 + w],
                                             ps[:, :w], fn,
                                             bias=lu_b[:, gi:gi + 1],
                                             scale=INV_G)
                    pe_warm(2)

                lgate(0); lgate(1)
                for (c0, w) in LIT_CH:
                    f0 = (c0 + 400) % 800
                    elem_chunk(lgates, c_lit, lit_gin, NLIT, c0, f0, w,
                               warm=8)

            # ---- vote head: mean over literals (sum on device; the flip
            # permutation of columns does not change the sum) ----
            v1 = work.tile([128, NLIT], BF16, tag="v1", name="v1", bufs=1)
            v2 = work.tile([128, NLIT], BF16, tag="v2", name="v2", bufs=1)
            for (c0, w) in LIT_CH:
                ps = ps_m()
                nc.tensor.matmul(ps[:, :w], vw0T[:],
                                 lit_gin[:, 1, c0:c0 + w],
                                 start=True, stop=True)
                nc.scalar.activation(v1[:, c0:c0 + w], ps[:, :w], AF.Relu,
                                     bias=vb[:, 0:1], scale=1.0 / S_H)
            for (c0, w) in LIT_CH:
                ps = ps_m()
                nc.tensor.matmul(ps[:, :w], vw1T[:], v1[:, c0:c0 + w],
                                 start=True, stop=True)
                nc.scalar.activation(v2[:, c0:c0 + w], ps[:, :w], AF.Relu,
                                     bias=vb[:, 1:2])
            acc = work.tile([1, 2], F32, tag="acc", name="acc", bufs=1)
            for ci, (c0, w) in enumerate(LIT_CH):
                ps = ps_m()
                nc.tensor.matmul(ps[0:1, :w], vw2T[:], v2[:, c0:c0 + w],
                                 start=True, stop=True)
                nc.vector.reduce_sum(acc[:, ci:ci + 1], ps[0:1, :w],
                                     axis=mybir.AxisListType.X)
            total = work.tile([1, 1], F32, tag="total", name="total", bufs=1)
            nc.vector.tensor_add(total[:], acc[:, 0:1], acc[:, 1:2])
            nc.sync.dma_start(out=out_d[:], in_=total[:])

    nc.compile()
    return nc


def _f8(x, scale=1.0):
    y = np.asarray(x, np.float32) * scale
    return np.clip(y, -240.0, 240.0).astype(NPF8)


def prep_inputs(inputs):
    """Host-side prep: per-core input dicts from the full problem inputs."""
    f32 = np.float32
    edge_src = np.asarray(inputs["edge_src"]).reshape(NG, NCL * KLIT)
    edge_dst = np.asarray(inputs["edge_dst"]).reshape(NG, NCL * KLIT)

    lmsg_w = np.asarray(inputs["lmsg_w"], f32)
    lmsg_b = np.asarray(inputs["lmsg_b"], f32)
    cmsg_w = np.asarray(inputs["cmsg_w"], f32)
    cmsg_b = np.asarray(inputs["cmsg_b"], f32)

    # msg weight stack [in, layer, out]: layer1 raw, layer2 /128 (to undo
    # the 128x h scale carried into x1), layer3 *64 (message scale); the
    # L1 bias is 128x so x1 = 128*relu(W0@h + b0).
    def msgT(w):
        t = np.transpose(w, (2, 0, 1)).copy()
        t[:, 1, :] /= S_H
        t[:, 2, :] *= S_M
        return np.ascontiguousarray(t).astype(NPBF)

    lmsgT = msgT(lmsg_w)
    cmsgT = msgT(cmsg_w)
    lmsg_b01 = np.ascontiguousarray(lmsg_b[0:2].T * np.float32([[S_H, 1.0]]))
    cmsg_b01 = np.ascontiguousarray(cmsg_b[0:2].T * np.float32([[S_H, 1.0]]))
    # aggregation-side biases carry the 64x message scale
    aggc_b = np.ascontiguousarray((S_M * KLIT * lmsg_b[2])[:, None])

    def gateT(w):  # [512, din] -> [din, 4, 128]
        return np.transpose(np.asarray(w, f32).reshape(4, 128, -1), (2, 0, 1))

    cu_wihT = gateT(inputs["cu_wih"])          # [128, 4, 128]
    cu_whhT = gateT(inputs["cu_whh"])
    cu_dr = _f8(np.stack([cu_wihT * S_W, cu_whhT * (S_W * S_M / S_H)],
                         axis=2))              # [128, 4, 2, 128]
    cu_b = np.ascontiguousarray(
        (np.asarray(inputs["cu_bih"], f32)
         + np.asarray(inputs["cu_bhh"], f32)).reshape(4, 128).T)
    lu_wih = np.asarray(inputs["lu_wih"], f32)  # [512, 256]
    lu_wihTa = gateT(lu_wih[:, :128])
    lu_wihTb = gateT(lu_wih[:, 128:])
    lu_dr = _f8(np.stack([lu_wihTa * S_W, lu_wihTb * (S_W * S_M / S_H)],
                         axis=2))
    lu_whhT = _f8(gateT(inputs["lu_whh"]) * (S_W * S_M / S_H))
    lu_b = np.ascontiguousarray(
        (np.asarray(inputs["lu_bih"], f32)
         + np.asarray(inputs["lu_bhh"], f32)).reshape(4, 128).T)

    vw0T = np.asarray(inputs["vote_w0"], f32).T.astype(NPBF)
    vw1T = np.asarray(inputs["vote_w1"], f32).T.astype(NPBF)
    vw2T = np.asarray(inputs["vote_w2"], f32).T.astype(NPBF)
    vb = np.stack([np.asarray(inputs["vote_b0"], f32),
                   np.asarray(inputs["vote_b1"], f32)], axis=1)

    h0l = (np.asarray(inputs["L_init_w"], f32)[:, 0]
           + np.asarray(inputs["L_init_b"], f32))
    h0c = (np.asarray(inputs["C_init_w"], f32)[:, 0]
           + np.asarray(inputs["C_init_b"], f32))
    h0_lit = _f8(np.broadcast_to(h0l[:, None], (128, NLIT)), S_H)
    h0_cl = _f8(np.broadcast_to(h0c[:, None], (128, NCL)), S_H)

    cmsg_b2 = cmsg_b[2]

    in_maps = []
    for g in range(NG):
        src = edge_src[g] - g * NNG          # local literal ids [0, 800)
        dst = edge_dst[g] - g * NNG - NLIT   # local clause ids [0, 1200)
        A = np.zeros((LCH * 128, NCL), f32)
        np.add.at(A, (src, dst), 1.0)
        deg = A.sum(axis=1)[:NLIT]           # literal degrees
        Ach = A.reshape(LCH, 128, NCL)       # [chunk, row, clause]
        # DR pairs (0,1),(2,3),(4,5) + plain chunk 6
        a_lc_dr = _f8(np.ascontiguousarray(
            Ach[:6].reshape(3, 2, 128, NCL).transpose(2, 0, 1, 3)))
        a_lc6 = _f8(np.ascontiguousarray(Ach[6]))
        At = np.zeros((CCH * 128, NLIT), f32)
        At[:NCL] = A[:NLIT].T
        a_cl_dr = _f8(np.ascontiguousarray(
            At.reshape(5, 2, 128, NLIT).transpose(2, 0, 1, 3)))
        aggl_b = np.ascontiguousarray(S_M * np.outer(cmsg_b2, deg))

        in_maps.append(dict(
            a_lc_dr=a_lc_dr, a_lc6=a_lc6, a_cl_dr=a_cl_dr,
            h0_lit=h0_lit, h0_cl=h0_cl,
            lmsgT=lmsgT, lmsg_b=lmsg_b01, cmsgT=cmsgT, cmsg_b=cmsg_b01,
            aggc_b=aggc_b, aggl_b=aggl_b,
            cu_dr=cu_dr, cu_b=cu_b,
            lu_dr=lu_dr, lu_whhT=lu_whhT, lu_b=lu_b,
            vw0T=vw0T, vw1T=vw1T, vw2T=vw2T, vb=vb,
        ))
    return in_maps


_NC_CACHE = {}
LAST_RESULT = None


def kernel(**inputs):
    global LAST_RESULT
    key = "main"
    if key not in _NC_CACHE:
        _NC_CACHE[key] = build_nc()
    nc = _NC_CACHE[key]
    in_maps = prep_inputs(inputs)
    res = run_bass_kernel_spmd(nc, in_maps, list(range(NG)))
    LAST_RESULT = res
    vote_b2 = float(np.asarray(inputs["vote_b2"], np.float32)[0])
    n_vars = np.asarray(inputs["n_vars"]).astype(np.float32)
    sums = np.array([res.results[g]["out"][0, 0] for g in range(NG)],
                    np.float32)
    sums = sums + np.float32(NLIT * vote_b2)
    return (sums / (2.0 * n_vars)).astype(np.float32)


# revision 34
# speedup vs baseline: 1.0359x; 1.0149x over previous
"""NeuronSAT GNN message passing on 8 Trainium2 NeuronCores (fp8 edition).

Sharding: data-parallel over graphs - graph g lives entirely on core g.
All state (h, c), weights, and the per-graph bipartite incidence matrices
are SBUF-resident for all 26 rounds.

Layout: feature-major [128=D, nodes]. The literal<->clause aggregations and
the LSTM gate matmuls run in fp8e4m3 with MatmulPerfMode.DoubleRow, which
packs two 128-deep contractions into one PE pass (2 fp8 weights per cell):
- aggregation: incidence-matrix chunk PAIRS contract together
  (7 lit chunks -> 3 DR + 1 plain; 10 clause chunks -> 5 DR)
- gates: the wih/whh (or wihA/wihB) term pairs contract together; the rhs
  is a [128, 2, n] tile holding [agg | h] as contiguous blocks, written
  in place by the aggregation post-op and the LSTM h-update respectively.

All quantization scales are powers of two folded into host-precomputed
weights (lossless in float arithmetic): messages are stored at 64x, h at
128x (|h|<1 so 128|h|<240 never overflows e4m3), gate weights at 256x/128x
so every gate PSUM comes out at 16384x and one activation scale undoes it.
The literal flip (negation) stays a pure column-slice trick by storing
literal h in FLIPPED column order inside the gate-input tile.

Numerics were validated against the jax reference in sim.py: this exact
quantization assignment gives rel err ~6.2e-3 on hardware (budget 2e-2).

Schedule notes (evidence-driven, from perfetto/NTFF traces):
- The round is a serial chain of 8 phases (two LSTM sides x MLP/agg/
  gates/elem); engine assignment is balanced per phase: all msg-MLP
  bias+relu posts ride ScalarE (idle during MLP phases while DVE does the
  node-major casts), agg_c posts ride ScalarE, agg_l posts (full-tensor
  bias) and all LSTM elementwise ride DVE.
- The 176-wide clause tail chunk is processed FIRST so its
  gates/elem/MLP/cast chain completes early and the last node-major cast
  (which gates agg_l) lands sooner.
- pe_warm keeps the PE HAM activity window busy (idle >3.4us throttles
  the PE clock to 1.2GHz); counts are tuned so throttle stays ~10-20us
  total without crowding real matmuls out of the queue (measured optimum;
  more warm delays the round-boundary chain, less re-throttles).
- Pair-major aggregation orders and 2-bank-wide gate ACTs were tried and
  measured SLOWER (psum rotation serialization / delayed elem starts).
HW exec time: ~726us on trn2 (baseline bf16 kernel: ~878us).
"""

import sys

sys.path.insert(0, "/opt/trn_rl_repo")

import ml_dtypes
import numpy as np

import concourse.bacc as bacc
import concourse.mybir as mybir
import concourse.tile as tile
from concourse.tile import add_dep_helper
from concourse.bass_utils import run_bass_kernel_spmd

# Problem dims (fixed by the reference).
NG = 8          # graphs == cores
NV = 400        # vars per graph
NCL = 1200      # clauses per graph
KLIT = 5        # literals per clause
NLIT = 2 * NV   # 800 literal nodes per graph
NNG = NLIT + NCL  # 2000 nodes per graph
D = 128
ROUNDS = 26
LCH = 7         # literal 128-chunks (last has 32 rows)
CCH = 10        # clause 128-chunks (last has 48 rows)

F32 = mybir.dt.float32
BF16 = mybir.dt.bfloat16
F8 = mybir.dt.float8e4
NPF8 = ml_dtypes.float8_e4m3fn
NPBF = ml_dtypes.bfloat16
AF = mybir.ActivationFunctionType
ALU = mybir.AluOpType
DR = mybir.MatmulPerfMode.DoubleRow

# Quantization scales (powers of 2; folded into weights host-side).
S_M = 64.0     # messages (m_nm tiles hold 64*m)
S_H = 128.0    # h state (gin block1 holds 128*h; |h|<1 -> <=128 < 240)
S_W = 256.0    # gate weight block0 scale; block1 = S_W*S_M/S_H = 128
S_G = S_M * S_W          # gate psum scale = 16384
INV_G = 1.0 / S_G

# Clause columns: 512-aligned chunks (PSUM bank = 512 f32).
CL_CH = [(0, 512), (512, 512), (1024, 176)]
# Literal columns: 400-wide; flip partner of [0:400] is [400:800].
LIT_CH = [(0, 400), (400, 400)]


def build_nc(rounds=ROUNDS):
    nc = bacc.Bacc(None, target_bir_lowering=False)

    def din(name, shape, dt):
        return nc.declare_dram_parameter(name, list(shape), dt, isOutput=False)

    a_lc_dr_d = din("a_lc_dr", [128, 3, 2, NCL], F8)
    a_lc6_d = din("a_lc6", [128, NCL], F8)
    a_cl_dr_d = din("a_cl_dr", [128, 5, 2, NLIT], F8)
    h0_lit_d = din("h0_lit", [128, NLIT], F8)
    h0_cl_d = din("h0_cl", [128, NCL], F8)
    lmsgT_d = din("lmsgT", [128, 3, 128], BF16)
    lmsg_b_d = din("lmsg_b", [128, 2], F32)
    cmsgT_d = din("cmsgT", [128, 3, 128], BF16)
    cmsg_b_d = din("cmsg_b", [128, 2], F32)
    aggc_b_d = din("aggc_b", [128, 1], F32)
    aggl_b_d = din("aggl_b", [128, NLIT], F32)
    cu_dr_d = din("cu_dr", [128, 4, 2, 128], F8)
    cu_b_d = din("cu_b", [128, 4], F32)
    lu_dr_d = din("lu_dr", [128, 4, 2, 128], F8)
    lu_whhT_d = din("lu_whhT", [128, 4, 128], F8)
    lu_b_d = din("lu_b", [128, 4], F32)
    vw0T_d = din("vw0T", [128, 128], BF16)
    vw1T_d = din("vw1T", [128, 128], BF16)
    vw2T_d = din("vw2T", [128, 1], BF16)
    vb_d = din("vb", [128, 2], F32)

    out_d = nc.declare_dram_parameter("out", [1, 1], F32, isOutput=True)

    with tile.TileContext(nc) as tc:
        with tc.tile_pool(name="singles", bufs=1) as singles, \
             tc.tile_pool(name="work", bufs=2) as work, \
             tc.tile_pool(name="ps", bufs=2, space="PSUM") as psp:

            def load(name, shape, dram, dt):
                t = singles.tile(list(shape), dt, tag=name, name=name)
                nc.sync.dma_start(out=t[:], in_=dram[:])
                return t

            # Loads ordered by first use inside round 0 so compute can
            # begin while the bigger, later-needed tensors still stream in.
            lmsgT = load("lmsgT", [128, 3, 128], lmsgT_d, BF16)
            lmsg_b = load("lmsg_b", [128, 2], lmsg_b_d, F32)

            # Gate-input tiles: block0 = agg (64x, fp8), block1 = h (128x,
            # fp8). Lit block1 is stored in FLIPPED column order so the DR
            # rhs [agg_l | h_flip] is one natural 3D slice.
            cl_gin = singles.tile([128, 2, NCL], F8, tag="cl_gin",
                                  name="cl_gin")
            lit_gin = singles.tile([128, 2, NLIT], F8, tag="lit_gin",
                                   name="lit_gin")
            nc.sync.dma_start(out=lit_gin[:, 1, :], in_=h0_lit_d[:])
            nc.sync.dma_start(out=cl_gin[:, 1, :], in_=h0_cl_d[:])

            a_lc_dr = load("a_lc_dr", [128, 3, 2, NCL], a_lc_dr_d, F8)
            a_lc6 = load("a_lc6", [128, NCL], a_lc6_d, F8)
            aggc_b = load("aggc_b", [128, 1], aggc_b_d, F32)
            cu_dr = load("cu_dr", [128, 4, 2, 128], cu_dr_d, F8)
            cu_b = load("cu_b", [128, 4], cu_b_d, F32)
            cmsgT = load("cmsgT", [128, 3, 128], cmsgT_d, BF16)
            cmsg_b = load("cmsg_b", [128, 2], cmsg_b_d, F32)
            a_cl_dr = load("a_cl_dr", [128, 5, 2, NLIT], a_cl_dr_d, F8)
            aggl_b = load("aggl_b", [128, NLIT], aggl_b_d, F32)
            lu_dr = load("lu_dr", [128, 4, 2, 128], lu_dr_d, F8)
            lu_whhT = load("lu_whhT", [128, 4, 128], lu_whhT_d, F8)
            lu_b = load("lu_b", [128, 4], lu_b_d, F32)
            vw0T = load("vw0T", [128, 128], vw0T_d, BF16)
            vw1T = load("vw1T", [128, 128], vw1T_d, BF16)
            vw2T = load("vw2T", [128, 1], vw2T_d, BF16)
            vb = load("vb", [128, 2], vb_d, F32)

            c_lit = singles.tile([128, NLIT], BF16, tag="c_lit", name="c_lit")
            c_cl = singles.tile([128, NCL], BF16, tag="c_cl", name="c_cl")
            nc.vector.memset(c_lit[:], 0.0)
            nc.vector.memset(c_cl[:], 0.0)

            # Node-major message tiles (fp8, 64x). Fully zeroed once so the
            # never-written tail rows of the last chunks stay 0 (junk fp8
            # bytes could be NaN and 0*NaN = NaN in the DR contraction).
            m_nm = singles.tile([128, 8, 128], F8, tag="m_nmL", name="m_nmL")
            m2_nm = singles.tile([128, 10, 128], F8, tag="m_nmC",
                                 name="m_nmC")
            nc.vector.memset(m_nm[:], 0.0)
            nc.vector.memset(m2_nm[:], 0.0)

            def ps_g(w=512):
                return psp.tile([128, 512], F32, tag="pg", name="pg", bufs=4)

            def ps_m(w=512):
                return psp.tile([128, 512], F32, tag="pm", name="pm", bufs=3)

            pw = psp.tile([128, 256], F32, tag="pW", name="pW", bufs=1)
            warm_rhs = lmsgT[:].rearrange("p a b -> p (a b)")[:, 0:256]

            last_mm = [None]

            def mm(*args, **kw):
                inst = nc.tensor.matmul(*args, **kw)
                last_mm[0] = inst
                return inst

            def pe_warm(n):
                """Keep-warm matmuls (N=256 stream each): harmless PE work
                that keeps the HAM activity window busy so the clock gate
                stays at 2.4GHz. An ordering-only edge to the latest real
                matmul pins the burst at this program position."""
                for k in range(n):
                    d = nc.tensor.matmul(pw[:], vw0T[:], warm_rhs,
                                         start=True, stop=True)
                    if k == 0 and last_mm[0] is not None:
                        add_dep_helper(d.ins, last_mm[0].ins, sync=False,
                                       reason="pin keep-warm burst")

            # ---------------- msg MLP helpers ----------------
            def layer_chunk(dst, srcT, b_ap, c0, w, src_ap, eng="dve"):
                """One MLP layer chunk: matmul + bias/relu post. The post
                runs on ScalarE for layer 1 and DVE for layer 2 so that
                consecutive layers pipeline on different engines."""
                ps = ps_m()
                mm(ps[:, :w], srcT, src_ap, start=True, stop=True)
                if eng == "act":
                    nc.scalar.activation(dst[:, c0:c0 + w], ps[:, :w],
                                         AF.Relu, bias=b_ap)
                else:
                    nc.vector.tensor_scalar(dst[:, c0:c0 + w], ps[:, :w],
                                            b_ap, 0.0, op0=ALU.add,
                                            op1=ALU.max)
                pe_warm(1)

            def nm_group(m_t, x2, ncols, wT2s, g0, gn, eng="dve"):
                """Node-major last-layer chunks g0..g0+gn packed into one
                psum bank, one DVE copy out (cast to fp8; psum is 64x m).
                A final partial chunk (k<128) is copied separately over just
                its valid partitions - the bank's other partitions hold
                stale junk which must not land in the fp8 tile (it is read
                by full-128-partition DR matmuls; fp8 junk can be NaN)."""
                ps = ps_m()
                klast = min(128, ncols - 128 * (g0 + gn - 1))
                for i in range(g0, g0 + gn):
                    k = min(128, ncols - 128 * i)
                    mm(ps[:k, 128 * (i - g0):128 * (i - g0) + 128],
                       x2[:, 128 * i:128 * i + k],
                       wT2s, start=True, stop=True)
                ps3 = ps[:].rearrange("p (b c) -> p b c", c=128)
                nfull = gn if klast == 128 else gn - 1
                if nfull:
                    if eng == "act":
                        nc.scalar.activation(
                            m_t[:, g0:g0 + nfull, :], ps3[:, 0:nfull, :],
                            AF.Copy)
                    else:
                        nc.vector.tensor_copy(m_t[:, g0:g0 + nfull, :],
                                              ps3[:, 0:nfull, :])
                if klast < 128:
                    nc.vector.tensor_copy(
                        m_t[:klast, g0 + gn - 1:g0 + gn, :],
                        ps3[:klast, gn - 1:gn, :])
                pe_warm(1)

            for r in range(rounds):
                # ---- forward: literal message MLP ----
                # L1 reads h (fp8 128x) -> psum 128*(W0@h); bias is 128*b0
                # host-side so x1 = 128*relu(W0@h+b0); W1 is pre-divided by
                # 128 so L2 psum is back at 1x.
                x1 = work.tile([128, NLIT], BF16, tag="mx1", name="mx1",
                               bufs=1)
                x2 = work.tile([128, NLIT], BF16, tag="mx2", name="mx2",
                               bufs=1)
                for (c0, w) in LIT_CH:
                    f0 = (c0 + 400) % 800
                    layer_chunk(x1, lmsgT[:, 0, :], lmsg_b[:, 0:1], c0, w,
                                lit_gin[:, 1, f0:f0 + w], eng="act")
                for (c0, w) in LIT_CH:
                    layer_chunk(x2, lmsgT[:, 1, :], lmsg_b[:, 1:2], c0, w,
                                x1[:, c0:c0 + w], eng="act")
                for (g0, gn) in ((0, 2), (2, 2), (4, 3)):
                    nm_group(m_nm, x2, NLIT, lmsgT[:, 2, :], g0, gn)

                # ---- agg into clauses: 3 DR pairs + 1 plain (32 rows);
                # tail chunk first, posts on ScalarE (idle in this phase) ----
                for (c0, w) in (CL_CH[2], CL_CH[0], CL_CH[1]):
                    ps = ps_m()
                    for g in range(3):
                        mm(ps[:, :w], m_nm[:, 2 * g:2 * g + 2, :],
                           a_lc_dr[:, g, :, c0:c0 + w],
                           start=(g == 0), stop=False, perf_mode=DR)
                    mm(ps[:, :w], m_nm[:32, 6, :], a_lc6[:32, c0:c0 + w],
                       start=False, stop=True)
                    nc.scalar.activation(cl_gin[:, 0, c0:c0 + w], ps[:, :w],
                                         AF.Identity, bias=aggc_b[:, 0:1])
                    pe_warm(1)

                # ---- clause LSTM + C_msg MLP, chunk-major interleaved ----
                cgates = {gi: work.tile([128, NCL], BF16, tag=f"cg{gi}",
                                        name=f"cg{gi}", bufs=1)
                          for gi in range(4)}
                y1 = work.tile([128, NCL], BF16, tag="my1", name="my1",
                               bufs=1)
                y2 = work.tile([128, NCL], BF16, tag="my2", name="my2",
                               bufs=1)

                def cgate(ci):
                    c0, w = CL_CH[ci]
                    for gi in (1, 0, 2, 3):
                        ps = ps_g()
                        mm(ps[:, :w], cu_dr[:, gi, :, :],
                           cl_gin[:, :, c0:c0 + w],
                           start=True, stop=True, perf_mode=DR)
                        fn = AF.Tanh if gi == 2 else AF.Sigmoid
                        nc.scalar.activation(cgates[gi][:, c0:c0 + w],
                                             ps[:, :w], fn,
                                             bias=cu_b[:, gi:gi + 1],
                                             scale=INV_G)
                    pe_warm(2)

                def celem(ci):
                    c0, w = CL_CH[ci]
                    elem_chunk(cgates, c_cl, cl_gin, NCL, c0, c0, w, warm=7)

                def elem_chunk(gates, c_t, gin, n, c0, h0, w, warm):
                    i_ = gates[0][:, c0:c0 + w]
                    f_ = gates[1][:, c0:c0 + w]
                    g_ = gates[2][:, c0:c0 + w]
                    o_ = gates[3][:, c0:c0 + w]
                    cs = c_t[:, c0:c0 + w]
                    t1 = work.tile([128, 512], BF16, tag="t1", name="t1")
                    t2 = work.tile([128, 512], BF16, tag="t2", name="t2")
                    nc.vector.tensor_mul(t1[:, :w], f_, cs)
                    nc.vector.tensor_mul(t2[:, :w], i_, g_)
                    nc.vector.tensor_add(cs, t1[:, :w], t2[:, :w])
                    tc2 = work.tile([128, 512], BF16, tag="tc2", name="tc2")
                    nc.scalar.activation(tc2[:, :w], cs, AF.Tanh)
                    # h (128x, fp8) = (o * 128) * tanh(c2)
                    nc.vector.scalar_tensor_tensor(
                        out=gin[:, 1, h0:h0 + w], in0=o_, scalar=S_H,
                        in1=tc2[:, :w], op0=ALU.mult, op1=ALU.mult)
                    pe_warm(warm)

                def cL1(ci):
                    c0, w = CL_CH[ci]
                    layer_chunk(y1, cmsgT[:, 0, :], cmsg_b[:, 0:1], c0, w,
                                cl_gin[:, 1, c0:c0 + w], eng="act")

                def cL2(ci):
                    c0, w = CL_CH[ci]
                    layer_chunk(y2, cmsgT[:, 1, :], cmsg_b[:, 1:2], c0, w,
                                y1[:, c0:c0 + w], eng="act")

                def cNM(ci):
                    g0, gn = ((0, 2), (2, 2), (4, 2), (6, 2), (8, 2))[ci]
                    nm_group(m2_nm, y2, NCL, cmsgT[:, 2, :], g0, gn)

                cgate(2); cgate(0); celem(2); cgate(1); celem(0)
                cL1(2); celem(1); cL2(2); cL1(0); cNM(4); cL1(1)
                cL2(0); cL2(1); cNM(0); cNM(1); cNM(2); cNM(3)
                pe_warm(7)

                # ---- agg into literals: 5 DR pairs ----
                for (c0, w) in LIT_CH:
                    ps = ps_m()
                    for j in range(5):
                        mm(ps[:, :w], m2_nm[:, 2 * j:2 * j + 2, :],
                           a_cl_dr[:, j, :, c0:c0 + w],
                           start=(j == 0), stop=(j == 4), perf_mode=DR)
                    nc.vector.tensor_add(lit_gin[:, 0, c0:c0 + w],
                                         ps[:, :w], aggl_b[:, c0:c0 + w])
                    pe_warm(1)

                # ---- literal LSTM (gate matmuls all read old h and
                # precede every h write) ----
                lgates = {gi: work.tile([128, NLIT], BF16, tag=f"lg{gi}",
                                        name=f"lg{gi}", bufs=1)
                          for gi in range(4)}

                def lgate(ci):
                    c0, w = LIT_CH[ci]
                    f0 = (c0 + 400) % 800
                    for gi in (1, 0, 2, 3):
                        ps = ps_g()
                        # DR: wihA@agg_l + wihB@h_flip (one natural slice)
                        mm(ps[:, :w], lu_dr[:, gi, :, :],
                           lit_gin[:, :, c0:c0 + w],
                           start=True, stop=False, perf_mode=DR)
                        # plain fp8: whh@h (h of chunk c0 lives at f0)
                        mm(ps[:, :w], lu_whhT[:, gi, :],
                           lit_gin[:, 1, f0:f0 + w],
                           start=False, stop=True)
                        fn = AF.Tanh if gi == 2 else AF.Sigmoid
                        nc.scalar.activation(lgates[gi][:, c0:c0 + w],
                                             ps[:, :w], fn,
                                             bias=lu_b[:, gi:gi + 1],
                                             scale=INV_G)
                    pe_warm(2)

                lgate(0); lgate(1)
                for (c0, w) in LIT_CH:
                    f0 = (c0 + 400) % 800
                    elem_chunk(lgates, c_lit, lit_gin, NLIT, c0, f0, w,
                               warm=8)

            # ---- vote head: mean over literals (sum on device; the flip
            # permutation of columns does not change the sum) ----
            v1 = work.tile([128, NLIT], BF16, tag="v1", name="v1", bufs=1)
            v2 = work.tile([128, NLIT], BF16, tag="v2", name="v2", bufs=1)
            for (c0, w) in LIT_CH:
                ps = ps_m()
                nc.tensor.matmul(ps[:, :w], vw0T[:],
                                 lit_gin[:, 1, c0:c0 + w],
                                 start=True, stop=True)
                nc.scalar.activation(v1[:, c0:c0 + w], ps[:, :w], AF.Relu,
                                     bias=vb[:, 0:1], scale=1.0 / S_H)
            for (c0, w) in LIT_CH:
                ps = ps_m()
                nc.tensor.matmul(ps[:, :w], vw1T[:], v1[:, c0:c0 + w],
                                 start=True, stop=True)
                nc.scalar.activation(v2[:, c0:c0 + w], ps[:, :w], AF.Relu,
                                     bias=vb[:, 1:2])
            acc = work.tile([1, 2], F32, tag="acc", name="acc", bufs=1)
            for ci, (c0, w) in enumerate(LIT_CH):
                ps = ps_m()
                nc.tensor.matmul(ps[0:1, :w], vw2T[:], v2[:, c0:c0 + w],
                                 start=True, stop=True)
                nc.vector.reduce_sum(acc[:, ci:ci + 1], ps[0:1, :w],
                                     axis=mybir.AxisListType.X)
            total = work.tile([1, 1], F32, tag="total", name="total", bufs=1)
            nc.vector.tensor_add(total[:], acc[:, 0:1], acc[:, 1:2])
            nc.sync.dma_start(out=out_d[:], in_=total[:])

    nc.compile()
    return nc


def _f8(x, scale=1.0):
    y = np.asarray(x, np.float32) * scale
    return np.clip(y, -240.0, 240.0).astype(NPF8)


def prep_inputs(inputs):
    """Host-side prep: per-core input dicts from the full problem inputs."""
    f32 = np.float32
    edge_src = np.asarray(inputs["edge_src"]).reshape(NG, NCL * KLIT)
    edge_dst = np.asarray(inputs["edge_dst"]).reshape(NG, NCL * KLIT)

    lmsg_w = np.asarray(inputs["lmsg_w"], f32)
    lmsg_b = np.asarray(inputs["lmsg_b"], f32)
    cmsg_w = np.asarray(inputs["cmsg_w"], f32)
    cmsg_b = np.asarray(inputs["cmsg_b"], f32)

    # msg weight stack [in, layer, out]: layer1 raw, layer2 /128 (to undo
    # the 128x h scale carried into x1), layer3 *64 (message scale); the
    # L1 bias is 128x so x1 = 128*relu(W0@h + b0).
    def msgT(w):
        t = np.transpose(w, (2, 0, 1)).copy()
        t[:, 1, :] /= S_H
        t[:, 2, :] *= S_M
        return np.ascontiguousarray(t).astype(NPBF)

    lmsgT = msgT(lmsg_w)
    cmsgT = msgT(cmsg_w)
    lmsg_b01 = np.ascontiguousarray(lmsg_b[0:2].T * np.float32([[S_H, 1.0]]))
    cmsg_b01 = np.ascontiguousarray(cmsg_b[0:2].T * np.float32([[S_H, 1.0]]))
    # aggregation-side biases carry the 64x message scale
    aggc_b = np.ascontiguousarray((S_M * KLIT * lmsg_b[2])[:, None])

    def gateT(w):  # [512, din] -> [din, 4, 128]
        return np.transpose(np.asarray(w, f32).reshape(4, 128, -1), (2, 0, 1))

    cu_wihT = gateT(inputs["cu_wih"])          # [128, 4, 128]
    cu_whhT = gateT(inputs["cu_whh"])
    cu_dr = _f8(np.stack([cu_wihT * S_W, cu_whhT * (S_W * S_M / S_H)],
                         axis=2))              # [128, 4, 2, 128]
    cu_b = np.ascontiguousarray(
        (np.asarray(inputs["cu_bih"], f32)
         + np.asarray(inputs["cu_bhh"], f32)).reshape(4, 128).T)
    lu_wih = np.asarray(inputs["lu_wih"], f32)  # [512, 256]
    lu_wihTa = gateT(lu_wih[:, :128])
    lu_wihTb = gateT(lu_wih[:, 128:])
    lu_dr = _f8(np.stack([lu_wihTa * S_W, lu_wihTb * (S_W * S_M / S_H)],
                         axis=2))
    lu_whhT = _f8(gateT(inputs["lu_whh"]) * (S_W * S_M / S_H))
    lu_b = np.ascontiguousarray(
        (np.asarray(inputs["lu_bih"], f32)
         + np.asarray(inputs["lu_bhh"], f32)).reshape(4, 128).T)

    vw0T = np.asarray(inputs["vote_w0"], f32).T.astype(NPBF)
    vw1T = np.asarray(inputs["vote_w1"], f32).T.astype(NPBF)
    vw2T = np.asarray(inputs["vote_w2"], f32).T.astype(NPBF)
    vb = np.stack([np.asarray(inputs["vote_b0"], f32),
                   np.asarray(inputs["vote_b1"], f32)], axis=1)

    h0l = (np.asarray(inputs["L_init_w"], f32)[:, 0]
           + np.asarray(inputs["L_init_b"], f32))
    h0c = (np.asarray(inputs["C_init_w"], f32)[:, 0]
           + np.asarray(inputs["C_init_b"], f32))
    h0_lit = _f8(np.broadcast_to(h0l[:, None], (128, NLIT)), S_H)
    h0_cl = _f8(np.broadcast_to(h0c[:, None], (128, NCL)), S_H)

    cmsg_b2 = cmsg_b[2]

    in_maps = []
    for g in range(NG):
        src = edge_src[g] - g * NNG          # local literal ids [0, 800)
        dst = edge_dst[g] - g * NNG - NLIT   # local clause ids [0, 1200)
        A = np.zeros((LCH * 128, NCL), f32)
        np.add.at(A, (src, dst), 1.0)
        deg = A.sum(axis=1)[:NLIT]           # literal degrees
        Ach = A.reshape(LCH, 128, NCL)       # [chunk, row, clause]
        # DR pairs (0,1),(2,3),(4,5) + plain chunk 6
        a_lc_dr = _f8(np.ascontiguousarray(
            Ach[:6].reshape(3, 2, 128, NCL).transpose(2, 0, 1, 3)))
        a_lc6 = _f8(np.ascontiguousarray(Ach[6]))
        At = np.zeros((CCH * 128, NLIT), f32)
        At[:NCL] = A[:NLIT].T
        a_cl_dr = _f8(np.ascontiguousarray(
            At.reshape(5, 2, 128, NLIT).transpose(2, 0, 1, 3)))
        aggl_b = np.ascontiguousarray(S_M * np.outer(cmsg_b2, deg))

        in_maps.append(dict(
            a_lc_dr=a_lc_dr, a_lc6=a_lc6, a_cl_dr=a_cl_dr,
            h0_lit=h0_lit, h0_cl=h0_cl,
            lmsgT=lmsgT, lmsg_b=lmsg_b01, cmsgT=cmsgT, cmsg_b=cmsg_b01,
            aggc_b=aggc_b, aggl_b=aggl_b,
            cu_dr=cu_dr, cu_b=cu_b,
            lu_dr=lu_dr, lu_whhT=lu_whhT, lu_b=lu_b,
            vw0T=vw0T, vw1T=vw1T, vw2T=vw2T, vb=vb,
        ))
    return in_maps


_NC_CACHE = {}
LAST_RESULT = None


def kernel(**inputs):
    global LAST_RESULT
    key = "main"
    if key not in _NC_CACHE:
        _NC_CACHE[key] = build_nc()
    nc = _NC_CACHE[key]
    in_maps = prep_inputs(inputs)
    res = run_bass_kernel_spmd(nc, in_maps, list(range(NG)))
    LAST_RESULT = res
    vote_b2 = float(np.asarray(inputs["vote_b2"], np.float32)[0])
    n_vars = np.asarray(inputs["n_vars"]).astype(np.float32)
    sums = np.array([res.results[g]["out"][0, 0] for g in range(NG)],
                    np.float32)
    sums = sums + np.float32(NLIT * vote_b2)
    return (sums / (2.0 * n_vars)).astype(np.float32)
